# revision 10
# baseline (speedup 1.0000x reference)
"""MoE (shared expert + 8 routed experts, top-2) on 8 Trainium2 NeuronCores.

Sharding: core c holds
  - shared-expert slice c: rows [c*1024, (c+1)*1024) of sw1/sw2 and the
    matching columns of sw3  -> partial (T, D) output, summed on host
  - routed expert c's weights (w12[c], w3[c]); host routes/gathers the
    tokens selected for expert c (capacity 1024 = the exact mean load),
    device computes unscaled expert outputs, host applies combine weights
    during the fp32 scatter-add; small per-expert overflows beyond the
    capacity are fixed up on host in fp32.

Device math is bf16 with fp32 PSUM accumulation; outputs are written
bf16 and promoted to fp32 during the host-side reduce.

v2 schedule: the routed expert runs FIRST (its 16.8MB of weights stream
at t=0 when the DMA queue is otherwise idle, with the contraction loop
ordered so matmuls start after ~0.5MB has landed); the shared-expert
weights prefetch behind routed compute so the phase transition has no
DMA bubble.  The v1 schedule ran shared-first and paid a ~30us stall +
HAM cold-clock window when the routed weights all loaded at the end.
"""

import sys

if "/opt/trn_rl_repo" not in sys.path:
    sys.path.insert(0, "/opt/trn_rl_repo")

from contextlib import ExitStack

import numpy as np
import ml_dtypes

import concourse.bass as bass
import concourse.tile as tile
from concourse import mybir, bacc
from concourse.bass_utils import run_bass_kernel_spmd

BF16 = mybir.dt.bfloat16
F32 = mybir.dt.float32
AF = mybir.ActivationFunctionType

# Problem shape (hardcoded per spec)
B, S, D = 2, 2048, 2048
T = B * S                  # 4096 tokens
E = 8                      # routed experts == n_cores
TOPK = 2
H_SHARED = 8192
HC = H_SHARED // 8         # shared-expert hidden slice per core
HR = 1024                  # routed expert hidden
NCORES = 8
NT = 512                   # token block (one PSUM bank at fp32)
P = 128
CH = 1024                  # shared-phase x chunk (2 sub-blocks)


def _build_program(C: int):
    """SPMD Bass program, routed capacity C (multiple of 128).

    Routed phase first, then shared; shared weights prefetch during the
    routed phase when C <= 1024 (SBUF budget), else after it.
    """
    nc = bacc.Bacc("TRN2", target_bir_lowering=False, debug=False)

    xT = nc.dram_tensor("xT", [D, T], BF16, kind="ExternalInput")
    sw1T = nc.dram_tensor("sw1T", [D, HC], BF16, kind="ExternalInput")
    sw2T = nc.dram_tensor("sw2T", [D, HC], BF16, kind="ExternalInput")
    sw3T = nc.dram_tensor("sw3T", [HC, D], BF16, kind="ExternalInput")
    # w12rT columns: [gate m0-3 | up m0-3 | gate m4-7 | up m4-7]
    w12rT = nc.dram_tensor("w12rT", [D, 2 * HR], BF16, kind="ExternalInput")
    w3T = nc.dram_tensor("w3T", [HR, D], BF16, kind="ExternalInput")
    xgT = nc.dram_tensor("xgT", [D, C], BF16, kind="ExternalInput")

    shared_outT = nc.dram_tensor("shared_outT", [D, T], BF16, kind="ExternalOutput")
    routed_outT = nc.dram_tensor("routed_outT", [D, C], BF16, kind="ExternalOutput")

    KD = D // P    # 16 contraction tiles over D
    KH = HC // P   # 8 contraction tiles over HC (== HR // P)
    H2 = HC // 2   # shared up-proj weight half-tile width

    # routed token blocks
    blk_w = [NT] * (C // NT)
    if C % NT:
        blk_w.append(C % NT)
    NBLK = len(blk_w)
    early_prefetch = C <= 1024

    with tile.TileContext(nc) as tc:
        with ExitStack() as ctx:
            # pools that live across both phases
            hpool = ctx.enter_context(tc.tile_pool(name="h_p", bufs=2))
            tpool = ctx.enter_context(tc.tile_pool(name="t_p", bufs=2))
            opool = ctx.enter_context(
                tc.tile_pool(name="o_p", bufs=4 if C <= 1024 else 2))
            # shared-phase first-half weights + chunk-0/sb-0 x: entered
            # before the routed pools (pool release is LIFO) so they
            # survive into the shared phase
            wsh1 = ctx.enter_context(tc.tile_pool(name="w_sh1", bufs=1))
            xA0 = ctx.enter_context(tc.tile_pool(name="x_a0", bufs=1))
            sw1_h1 = [wsh1.tile([P, H2], BF16, name=f"sw1_{k}_0",
                                tag=f"sw1_{k}_0") for k in range(KD)]
            sw2_h1 = [wsh1.tile([P, H2], BF16, name=f"sw2_{k}_0",
                                tag=f"sw2_{k}_0") for k in range(KD)]
            x00 = [xA0.tile([P, NT], BF16, name=f"xa_{k}", tag=f"xa_{k}")
                   for k in range(KD)]

            # TRN2 has two HWDGE queues (sync + scalar).  All these loads
            # write freshly-allocated SBUF (no WAR waits), so they can ride
            # either queue.  Scalar takes only sw2 so its queue drains
            # before the routed silus need it (~32us in).
            def emit_prefetch_a():
                for k in range(KD):
                    nc.sync.dma_start(sw1_h1[k][:], sw1T[k * P:(k + 1) * P, :H2])
                    nc.scalar.dma_start(sw2_h1[k][:], sw2T[k * P:(k + 1) * P, :H2])
                for k in range(KD):
                    nc.sync.dma_start(x00[k][:], xT[k * P:(k + 1) * P, 0:NT])

            # HAM warmup: dummy matmuls on uninitialized SBUF (no deps, so
            # they issue right after the engine preamble) keep the PE busy
            # through its cold-clock window while the first weights stream
            # in; without them the first ~3.4us of real matmuls run at
            # half clock. Results land in a scratch PSUM bank, never read.
            wmp = ctx.enter_context(tc.tile_pool(name="wm_p", bufs=1))
            wwm = wmp.tile([P, P], BF16, name="wwm", tag="wwm")
            xwm = wmp.tile([P, NT // 2], BF16, name="xwm", tag="xwm")
            nc.vector.memset(wwm[:], 0)
            nc.gpsimd.memset(xwm[:], 0)

            with ExitStack() as ctx_r:
                # ---------------- Phase R: routed expert ----------------
                wr = ctx_r.enter_context(tc.tile_pool(name="w_r", bufs=1))
                xgp = ctx_r.enter_context(tc.tile_pool(name="xg_p", bufs=1))
                psR = ctx_r.enter_context(
                    tc.tile_pool(name="psR", bufs=1, space="PSUM"))

                for g in range(2):
                    pw = psR.tile([P, NT], F32, name="pb7", tag="pb7")
                    for k in range(6):
                        nc.tensor.matmul(pw[:, :NT // 2], wwm[:], xwm[:],
                                         start=(k == 0), stop=(k == 5))

                # full-width tiles, one DMA each: the sync engine issues
                # DMA instructions serially (~0.6us apiece, ring depth 1),
                # so fewer/bigger transfers beat many small ones
                w12_sb = [wr.tile([P, 2 * HR], BF16, name=f"w12_{k}",
                                  tag=f"w12_{k}") for k in range(KD)]
                w3_sb = [wr.tile([P, D], BF16, name=f"w3_{k}", tag=f"w3_{k}")
                         for k in range(KH)]
                xg_sb = [xgp.tile([P, C], BF16, name=f"xg_{k}", tag=f"xg_{k}")
                         for k in range(KD)]

                # DMA emission = execution order, need-ordered.  The two
                # HWDGE queues issue in parallel: scalar carries the mg0
                # weight halves (the first matmul wave's gating bytes),
                # sync carries the tokens + everything later.  Scalar's
                # stream is sized to drain before the first routed silu
                # (~32us) so the activation queue never backs up.
                nt0 = min(NT, C)
                for k in range(KD):
                    if k < 2:
                        # first weight tiles split small: the first
                        # matmuls' dependencies land ~1us sooner
                        nc.scalar.dma_start(w12_sb[k][:, 0:NT],
                                            w12rT[k * P:(k + 1) * P, 0:NT])
                        nc.sync.dma_start(xg_sb[k][:, 0:nt0],
                                          xgT[k * P:(k + 1) * P, 0:nt0])
                        nc.scalar.dma_start(w12_sb[k][:, NT:HR],
                                            w12rT[k * P:(k + 1) * P, NT:HR])
                    else:
                        nc.sync.dma_start(xg_sb[k][:, 0:nt0],
                                          xgT[k * P:(k + 1) * P, 0:nt0])
                        nc.scalar.dma_start(w12_sb[k][:, 0:HR],
                                            w12rT[k * P:(k + 1) * P, 0:HR])
                for k in range(KD):
                    nc.sync.dma_start(w12_sb[k][:, HR:],
                                      w12rT[k * P:(k + 1) * P, HR:])
                for k in range(KH):
                    nc.sync.dma_start(w3_sb[k][:], w3T[k * P:(k + 1) * P, :])
                if C > nt0:
                    for k in range(KD):
                        nc.sync.dma_start(xg_sb[k][:, nt0:],
                                          xgT[k * P:(k + 1) * P, nt0:])

                if early_prefetch:
                    # shared-phase critical prefetch streams behind the
                    # routed loads
                    emit_prefetch_a()

                # routed compute
                off = 0
                for b, nt in enumerate(blk_w):
                    tok = slice(off, off + nt)
                    off += nt
                    hs = []
                    for mg in range(2):
                        pg = [psR.tile([P, NT], F32, name=f"pb{m}", tag=f"pb{m}")
                              for m in range(4)]
                        pu = [psR.tile([P, NT], F32, name=f"pb{4+m}", tag=f"pb{4+m}")
                              for m in range(4)]
                        # k emitted in quarters so the first matmuls only
                        # depend on the first few weight tiles
                        for kq in range(4):
                            ks = range(kq * 4, kq * 4 + 4)
                            for m in range(4):
                                # gate / up columns inside the mg half
                                c1 = slice(mg * HR + m * P, mg * HR + (m + 1) * P)
                                c2 = slice(mg * HR + NT + m * P,
                                           mg * HR + NT + (m + 1) * P)
                                for k in ks:
                                    nc.tensor.matmul(pg[m][:, :nt],
                                                     w12_sb[k][:, c1],
                                                     xg_sb[k][:, tok],
                                                     start=(k == 0), stop=(k == KD - 1))
                                for k in ks:
                                    nc.tensor.matmul(pu[m][:, :nt],
                                                     w12_sb[k][:, c2],
                                                     xg_sb[k][:, tok],
                                                     start=(k == 0), stop=(k == KD - 1))
                        for m in range(4):
                            sg = tpool.tile([P, NT], F32, name="sg", tag="sg")
                            nc.scalar.activation(sg[:, :nt], pg[m][:, :nt], AF.Silu)
                            h = hpool.tile([P, NT], BF16, name=f"h_{mg*4+m}",
                                           tag=f"h_{mg*4+m}")
                            nc.vector.tensor_mul(h[:, :nt], sg[:, :nt], pu[m][:, :nt])
                            hs.append(h)
                    last_blk = (b == NBLK - 1)
                    for mo in range(KD):
                        po = psR.tile([P, NT], F32, name=f"pb{mo%8}", tag=f"pb{mo%8}")
                        for k in range(KH):
                            nc.tensor.matmul(po[:, :nt],
                                             w3_sb[k][:, mo * P:(mo + 1) * P],
                                             hs[k][:, :nt],
                                             start=(k == 0), stop=(k == KH - 1))
                        so = opool.tile([P, NT], BF16, name="so", tag="so")
                        if last_blk and mo % 2 == 1:
                            # alternate the PSUM-drain copies between the two
                            # PSUM-capable engines (DVE + Act) so every PSUM
                            # bank's WAR (blocking the shared phase's first
                            # matmuls) clears right after that bank's
                            # matmuls, not after a serial 16-copy DVE backlog
                            nc.scalar.activation(so[:, :nt], po[:, :nt],
                                                 AF.Copy)
                        else:
                            nc.vector.tensor_copy(so[:, :nt], po[:, :nt])
                        # outputs ride the SWDGE so the sync queue keeps
                        # streaming weights
                        nc.gpsimd.dma_start(routed_outT[mo * P:(mo + 1) * P, tok],
                                            so[:, :nt])

            # ---------------- Phase S: shared expert ----------------
            wsh2 = ctx.enter_context(tc.tile_pool(name="w_sh2", bufs=1))
            wdn = ctx.enter_context(tc.tile_pool(name="w_dn", bufs=1))
            xpool = ctx.enter_context(tc.tile_pool(name="x_p", bufs=2))
            psA = ctx.enter_context(tc.tile_pool(name="psA", bufs=2, space="PSUM"))
            psB = ctx.enter_context(tc.tile_pool(name="psB", bufs=4, space="PSUM"))

            if not early_prefetch:
                emit_prefetch_a()

            # second weight halves, down-proj weights, chunk-0/sb-1 x:
            # these land in SBUF freed by the routed pools (WAR-ordered).
            # sw3 and x01 interleave so neither arrives at the wire.
            sw1_h2 = [wsh2.tile([P, H2], BF16, name=f"sw1_{k}_1", tag=f"sw1_{k}_1")
                      for k in range(KD)]
            sw2_h2 = [wsh2.tile([P, H2], BF16, name=f"sw2_{k}_1", tag=f"sw2_{k}_1")
                      for k in range(KD)]
            sw3_sb = [wdn.tile([P, D], BF16, name=f"sw3_{k}", tag=f"sw3_{k}")
                      for k in range(KH)]
            x01 = [xpool.tile([P, NT], BF16, name=f"x_{k}_1", tag=f"x_{k}_1")
                   for k in range(KD)]
            for k in range(KD):
                nc.sync.dma_start(sw1_h2[k][:], sw1T[k * P:(k + 1) * P, H2:])
                nc.sync.dma_start(sw2_h2[k][:], sw2T[k * P:(k + 1) * P, H2:])
            for k in range(KH // 2):
                nc.sync.dma_start(sw3_sb[k][:], sw3T[k * P:(k + 1) * P, :])
            for k in range(KD // 2):
                nc.sync.dma_start(x01[k][:], xT[k * P:(k + 1) * P, NT:CH])
            for k in range(KH // 2, KH):
                nc.sync.dma_start(sw3_sb[k][:], sw3T[k * P:(k + 1) * P, :])
            for k in range(KD // 2, KD):
                nc.sync.dma_start(x01[k][:], xT[k * P:(k + 1) * P, NT:CH])

            sw_h = [[sw1_h1, sw1_h2], [sw2_h1, sw2_h2]]

            for ch in range(T // CH):
                if ch == 0:
                    x_sb = [[x00[k], x01[k]] for k in range(KD)]
                else:
                    x_sb = [[xpool.tile([P, NT], BF16, name=f"x_{k}_{h}",
                                        tag=f"x_{k}_{h}")
                             for h in range(CH // NT)] for k in range(KD)]
                    for k in range(KD):
                        for h in range(CH // NT):
                            nc.sync.dma_start(
                                x_sb[k][h][:],
                                xT[k * P:(k + 1) * P,
                                   ch * CH + h * NT:ch * CH + (h + 1) * NT])
                for sb in range(CH // NT):
                    otok = slice(ch * CH + sb * NT, ch * CH + (sb + 1) * NT)
                    hs = []
                    for m in range(KH):
                        wh, wm = divmod(m, H2 // P)   # which weight half-tile
                        mm = slice(wm * P, (wm + 1) * P)
                        pg = psA.tile([P, NT], F32, name="pg", tag="pg")
                        pu = psA.tile([P, NT], F32, name="pu", tag="pu")
                        for k in range(KD):
                            nc.tensor.matmul(pg[:], sw_h[0][wh][k][:, mm],
                                             x_sb[k][sb][:],
                                             start=(k == 0), stop=(k == KD - 1))
                        for k in range(KD):
                            nc.tensor.matmul(pu[:], sw_h[1][wh][k][:, mm],
                                             x_sb[k][sb][:],
                                             start=(k == 0), stop=(k == KD - 1))
                        sg = tpool.tile([P, NT], F32, name="sg", tag="sg")
                        nc.scalar.activation(sg[:], pg[:], AF.Silu)
                        h = hpool.tile([P, NT], BF16, name=f"h_{m}", tag=f"h_{m}")
                        nc.vector.tensor_mul(h[:], sg[:], pu[:])
                        hs.append(h)
                    last_sb = (ch == T // CH - 1 and sb == CH // NT - 1)
                    for mo in range(KD):
                        orow = slice(mo * P, (mo + 1) * P)
                        if last_sb and mo >= KD - 2:
                            # final two tiles in half-width groups (separate
                            # PSUM banks), copies rotated across engines and
                            # DMAs split over both idle HWDGE queues, so the
                            # post-matmul drain is one [128,256] copy + one
                            # cheap SWDGE issue
                            cp_eng = [nc.vector, nc.scalar, nc.vector,
                                      nc.scalar]
                            dma_eng = [nc.sync, nc.sync, nc.sync, nc.gpsimd]
                            for hf in range(2):
                                pi = (mo - (KD - 2)) * 2 + hf
                                cs = slice(hf * (NT // 2), (hf + 1) * (NT // 2))
                                po = psB.tile([P, NT], F32, name="po", tag="po")
                                for k in range(KH):
                                    nc.tensor.matmul(po[:, :NT // 2],
                                                     sw3_sb[k][:, orow],
                                                     hs[k][:, cs],
                                                     start=(k == 0), stop=(k == KH - 1))
                                so = opool.tile([P, NT], BF16, name="so", tag="so")
                                ce = cp_eng[pi]
                                if ce is nc.scalar:
                                    nc.scalar.activation(so[:, :NT // 2],
                                                         po[:, :NT // 2], AF.Copy)
                                else:
                                    ce.tensor_copy(so[:, :NT // 2], po[:, :NT // 2])
                                dma_eng[pi].dma_start(
                                    shared_outT[orow,
                                                otok.start + hf * (NT // 2):
                                                otok.start + (hf + 1) * (NT // 2)],
                                    so[:, :NT // 2])
                            continue
                        po = psB.tile([P, NT], F32, name="po", tag="po")
                        for k in range(KH):
                            nc.tensor.matmul(po[:], sw3_sb[k][:, orow],
                                             hs[k][:],
                                             start=(k == 0), stop=(k == KH - 1))
                        so = opool.tile([P, NT], BF16, name="so", tag="so")
                        if last_sb:
                            # alternate drain copies between the PSUM-capable
                            # engines + split output DMAs over both HWDGE
                            # queues and the SWDGE
                            if mo % 2 == 1:
                                nc.scalar.activation(so[:], po[:], AF.Copy)
                            else:
                                nc.vector.tensor_copy(so[:], po[:])
                            (nc.sync, nc.gpsimd, nc.scalar)[mo % 3].dma_start(
                                shared_outT[orow, otok], so[:])
                        else:
                            nc.vector.tensor_copy(so[:], po[:])
                            nc.gpsimd.dma_start(shared_outT[orow, otok], so[:])

    nc.compile()
    return nc


_PROGRAM_CACHE: dict = {}


def _get_program(C: int):
    if C not in _PROGRAM_CACHE:
        _PROGRAM_CACHE[C] = _build_program(C)
    return _PROGRAM_CACHE[C]


def _route_like_reference(xf: np.ndarray, router_w: np.ndarray,
                          expert_bias: np.ndarray):
    """Router computed with jax on CPU to bit-match the reference's top-k."""
    import jax
    import jax.numpy as jnp

    cpu = jax.devices("cpu")[0]
    with jax.default_device(cpu):
        xj = jnp.asarray(xf)
        scores = jax.nn.sigmoid(xj @ jnp.asarray(router_w).T)        # (T, E)
        sel = scores + jnp.asarray(expert_bias)
        _, top_idx = jax.lax.top_k(sel, TOPK)                        # (T, K)
        top_sc = jnp.take_along_axis(scores, top_idx, axis=-1)
        top_w = top_sc / (top_sc.sum(-1, keepdims=True) + 1e-9)
        return np.asarray(top_idx), np.asarray(top_w)


def kernel(x, w12, w3, router_w, expert_bias, sw1, sw2, sw3):
    x = np.asarray(x, dtype=np.float32)
    w12 = np.asarray(w12, dtype=np.float32)
    w3 = np.asarray(w3, dtype=np.float32)
    router_w = np.asarray(router_w, dtype=np.float32)
    expert_bias = np.asarray(expert_bias, dtype=np.float32)
    sw1 = np.asarray(sw1, dtype=np.float32)
    sw2 = np.asarray(sw2, dtype=np.float32)
    sw3 = np.asarray(sw3, dtype=np.float32)

    xf = x.reshape(T, D)
    top_idx, top_w = _route_like_reference(xf, router_w, expert_bias)

    # per-expert token lists + combine weights
    idx_list, w_list = [], []
    for e in range(E):
        hit = top_idx == e                      # (T, K)
        tok = np.nonzero(hit.any(axis=1))[0]
        wt = (top_w * hit).sum(axis=1)[tok]     # combine weight per token
        idx_list.append(tok.astype(np.int64))
        w_list.append(wt.astype(np.float32))

    max_n = max(len(i) for i in idx_list)
    # Device capacity policy: cap at C_CORE (the exact mean load for top-2 of
    # 8 experts) and fix up small per-expert overflows on host in fp32
    # (<0.2% of FLOPs, like the router). Grossly imbalanced routing falls
    # back to extra device launches in slabs of C_MAX.
    C_CORE = 1024
    C_MAX = 1280   # slab size for the imbalanced-routing fallback (SBUF limit)
    overflow = sum(max(0, len(i) - C_CORE) for i in idx_list)
    if max_n <= C_CORE:
        C = max(P, -(-max_n // P) * P)          # capacity, multiple of 128
        n_launches, host_fix = 1, False
    elif overflow <= 1024:
        C, n_launches, host_fix = C_CORE, 1, True
    else:
        C = C_MAX
        n_launches, host_fix = max(1, -(-max_n // C_MAX)), False

    xT16 = np.ascontiguousarray(xf.T).astype(ml_dtypes.bfloat16)   # (D, T)

    nc = _get_program(C)

    sw_z = np.zeros((D, HC), dtype=ml_dtypes.bfloat16)
    sw3_z = np.zeros((HC, D), dtype=ml_dtypes.bfloat16)

    outT = np.zeros((D, T), dtype=np.float32)
    global _LAST_RESULTS
    for launch in range(n_launches):
        lo = launch * C_MAX
        in_maps = []
        for c in range(NCORES):
            hs = slice(c * HC, (c + 1) * HC)
            idx_c = idx_list[c][lo:lo + C]
            w_c = w_list[c][lo:lo + C]
            n_c = len(idx_c)
            xg = np.zeros((D, C), dtype=ml_dtypes.bfloat16)
            xg[:, :n_c] = xT16[:, idx_c]
            if launch == 0:
                s1 = np.ascontiguousarray(sw1[hs].T).astype(ml_dtypes.bfloat16)
                s2 = np.ascontiguousarray(sw2[hs].T).astype(ml_dtypes.bfloat16)
                s3 = np.ascontiguousarray(sw3[:, hs].T).astype(ml_dtypes.bfloat16)
            else:
                s1, s2, s3 = sw_z, sw_z, sw3_z   # shared part already done
            # reorder w12 columns into [gate m0-3 | up m0-3 | gate m4-7 | up m4-7]
            w12t = np.ascontiguousarray(w12[c].T).astype(ml_dtypes.bfloat16)
            w12r = np.concatenate([w12t[:, 0:NT], w12t[:, HR:HR + NT],
                                   w12t[:, NT:HR], w12t[:, HR + NT:]], axis=1)
            in_maps.append({
                "xT": xT16,
                "sw1T": s1, "sw2T": s2, "sw3T": s3,
                "w12rT": np.ascontiguousarray(w12r),
                "w3T": np.ascontiguousarray(w3[c].T).astype(ml_dtypes.bfloat16),
                "xgT": xg,
            })

        res = run_bass_kernel_spmd(nc, in_maps, core_ids=list(range(NCORES)),
                                   **_RUN_KWARGS)
        _LAST_RESULTS = res

        for c in range(NCORES):
            if launch == 0:
                outT += res.results[c]["shared_outT"].astype(np.float32)
            idx_c = idx_list[c][lo:lo + C]
            if len(idx_c):
                # token indices are unique within one expert; combine weight
                # applied here in fp32
                ro = res.results[c]["routed_outT"][:, :len(idx_c)].astype(np.float32)
                outT[:, idx_c] += ro * w_list[c][lo:lo + C][None, :]

    if host_fix:
        # fp32 fixup for tokens beyond the device capacity of each expert
        for c in range(NCORES):
            tail = idx_list[c][C:]
            if len(tail) == 0:
                continue
            wts = w_list[c][C:]
            xs = xf[tail]                             # (n, D)
            h12 = xs @ w12[c].T                       # (n, 2*HR)
            h1, h2 = h12[:, :HR], h12[:, HR:]
            h = h1 / (1.0 + np.exp(-h1)) * h2         # silu(h1) * h2
            out = (h * wts[:, None]) @ w3[c].T        # (n, D)
            outT[:, tail] += out.T
    return outT.T.reshape(B, S, D).astype(np.float32)


# test harness hooks: set _RUN_KWARGS = {"trace": True, ...} before calling
# kernel() to profile; read _LAST_RESULTS afterwards.
_RUN_KWARGS: dict = {}
_LAST_RESULTS = None



# revision 17
# speedup vs baseline: 1.0112x; 1.0112x over previous
"""MoE (shared expert + 8 routed experts, top-2) on 8 Trainium2 NeuronCores.

Sharding: core c holds
  - shared-expert slice c: rows [c*1024, (c+1)*1024) of sw1/sw2 and the
    matching columns of sw3  -> partial (T, D) output, summed on host
  - routed expert c's weights (w12[c], w3[c]); host routes/gathers the
    tokens selected for expert c (capacity 1024 = the exact mean load),
    device computes unscaled expert outputs, host applies combine weights
    during the fp32 scatter-add; small per-expert overflows beyond the
    capacity are fixed up on host in fp32.

Device math is bf16 with fp32 PSUM accumulation; outputs are written
bf16 and promoted to fp32 during the host-side reduce.

v2 schedule: the routed expert runs FIRST (its 16.8MB of weights stream
at t=0 when the DMA queue is otherwise idle, with the contraction loop
ordered so matmuls start after ~0.5MB has landed); the shared-expert
weights prefetch behind routed compute so the phase transition has no
DMA bubble.  The v1 schedule ran shared-first and paid a ~30us stall +
HAM cold-clock window when the routed weights all loaded at the end.
"""

import sys

if "/opt/trn_rl_repo" not in sys.path:
    sys.path.insert(0, "/opt/trn_rl_repo")

from contextlib import ExitStack

import numpy as np
import ml_dtypes

import concourse.bass as bass
import concourse.tile as tile
from concourse import mybir, bacc
from concourse.bass_utils import run_bass_kernel_spmd

BF16 = mybir.dt.bfloat16
F32 = mybir.dt.float32
AF = mybir.ActivationFunctionType

# Problem shape (hardcoded per spec)
B, S, D = 2, 2048, 2048
T = B * S                  # 4096 tokens
E = 8                      # routed experts == n_cores
TOPK = 2
H_SHARED = 8192
HC = H_SHARED // 8         # shared-expert hidden slice per core
HR = 1024                  # routed expert hidden
NCORES = 8
NT = 512                   # token block (one PSUM bank at fp32)
P = 128
CH = 1024                  # shared-phase x chunk (2 sub-blocks)


def _build_program(C: int):
    """SPMD Bass program, routed capacity C (multiple of 128).

    Routed phase first, then shared; shared weights prefetch during the
    routed phase when C <= 1024 (SBUF budget), else after it.
    """
    nc = bacc.Bacc("TRN2", target_bir_lowering=False, debug=False)

    xT = nc.dram_tensor("xT", [D, T], BF16, kind="ExternalInput")
    sw1T = nc.dram_tensor("sw1T", [D, HC], BF16, kind="ExternalInput")
    sw2T = nc.dram_tensor("sw2T", [D, HC], BF16, kind="ExternalInput")
    sw3T = nc.dram_tensor("sw3T", [HC, D], BF16, kind="ExternalInput")
    # w12rT columns: [gate m0-3 | up m0-3 | gate m4-7 | up m4-7]
    w12rT = nc.dram_tensor("w12rT", [D, 2 * HR], BF16, kind="ExternalInput")
    w3T = nc.dram_tensor("w3T", [HR, D], BF16, kind="ExternalInput")
    xgT = nc.dram_tensor("xgT", [D, C], BF16, kind="ExternalInput")

    shared_outT = nc.dram_tensor("shared_outT", [D, T], BF16, kind="ExternalOutput")
    routed_outT = nc.dram_tensor("routed_outT", [D, C], BF16, kind="ExternalOutput")

    KD = D // P    # 16 contraction tiles over D
    KH = HC // P   # 8 contraction tiles over HC (== HR // P)
    H2 = HC // 2   # shared up-proj weight half-tile width

    # routed token blocks
    blk_w = [NT] * (C // NT)
    if C % NT:
        blk_w.append(C % NT)
    NBLK = len(blk_w)
    early_prefetch = C <= 1024

    with tile.TileContext(nc) as tc:
        with ExitStack() as ctx:
            # pools that live across both phases
            hpool = ctx.enter_context(tc.tile_pool(name="h_p", bufs=2))
            tpool = ctx.enter_context(tc.tile_pool(name="t_p", bufs=2))
            opool = ctx.enter_context(
                tc.tile_pool(name="o_p", bufs=4 if C <= 1024 else 2))
            # shared-phase first-half weights + chunk-0/sb-0 x: entered
            # before the routed pools (pool release is LIFO) so they
            # survive into the shared phase
            wsh1 = ctx.enter_context(tc.tile_pool(name="w_sh1", bufs=1))
            xA0 = ctx.enter_context(tc.tile_pool(name="x_a0", bufs=1))
            sw1_h1 = [wsh1.tile([P, H2], BF16, name=f"sw1_{k}_0",
                                tag=f"sw1_{k}_0") for k in range(KD)]
            sw2_h1 = [wsh1.tile([P, H2], BF16, name=f"sw2_{k}_0",
                                tag=f"sw2_{k}_0") for k in range(KD)]
            x00 = [xA0.tile([P, NT], BF16, name=f"xa_{k}", tag=f"xa_{k}")
                   for k in range(KD)]

            # input DMAs stay on the sync queue alone: the stream is DMA-ring
            # completion-limited (~wire speed), so a second issue queue only
            # reorders arrivals, and the scalar queue must stay clear for the
            # silu chain (a backlog there stalls the PE on PSUM WAR)
            def emit_prefetch_a():
                for k in range(KD):
                    nc.sync.dma_start(sw1_h1[k][:], sw1T[k * P:(k + 1) * P, :H2])
                    nc.sync.dma_start(sw2_h1[k][:], sw2T[k * P:(k + 1) * P, :H2])
                for k in range(KD):
                    nc.sync.dma_start(x00[k][:], xT[k * P:(k + 1) * P, 0:NT])

            # HAM warmup: dummy matmuls on uninitialized SBUF (no deps, so
            # they issue right after the engine preamble) keep the PE busy
            # through its cold-clock window while the first weights stream
            # in; without them the first ~3.4us of real matmuls run at
            # half clock. Results land in a scratch PSUM bank, never read.
            wmp = ctx.enter_context(tc.tile_pool(name="wm_p", bufs=1))
            wwm = wmp.tile([P, P], BF16, name="wwm", tag="wwm")
            xwm = wmp.tile([P, NT // 2], BF16, name="xwm", tag="xwm")
            nc.vector.memset(wwm[:], 0)
            nc.gpsimd.memset(xwm[:], 0)

            with ExitStack() as ctx_r:
                # ---------------- Phase R: routed expert ----------------
                wr = ctx_r.enter_context(tc.tile_pool(name="w_r", bufs=1))
                xgp = ctx_r.enter_context(tc.tile_pool(name="xg_p", bufs=1))
                psR = ctx_r.enter_context(
                    tc.tile_pool(name="psR", bufs=1, space="PSUM"))

                for g in range(2):
                    pw = psR.tile([P, NT], F32, name="pb7", tag="pb7")
                    for k in range(5):
                        nc.tensor.matmul(pw[:, :NT // 2], wwm[:], xwm[:],
                                         start=(k == 0), stop=(k == 4))

                # full-width tiles, one DMA each: the sync engine issues
                # DMA instructions serially (~0.6us apiece, ring depth 1),
                # so fewer/bigger transfers beat many small ones
                w12_sb = [wr.tile([P, 2 * HR], BF16, name=f"w12_{k}",
                                  tag=f"w12_{k}") for k in range(KD)]
                w3_sb = [wr.tile([P, D], BF16, name=f"w3_{k}", tag=f"w3_{k}")
                         for k in range(KH)]
                xg_sb = [xgp.tile([P, C], BF16, name=f"xg_{k}", tag=f"xg_{k}")
                         for k in range(KD)]

                # DMA emission = execution order on the sync queue,
                # need-ordered: block-0 tokens + mg0 weight halves feed the
                # first matmul wave with exactly its bytes; then mg1
                # halves, then w3 (block-0 down-proj), then the remaining
                # tokens (block 1+)
                nt0 = min(NT, C)
                for k in range(KD):
                    if k < 2:
                        # first tiles split small across DMA rings: the
                        # first matmuls' dependencies land ~2us sooner
                        h0 = nt0 // 2
                        nc.sync.dma_start(xg_sb[k][:, 0:h0],
                                          xgT[k * P:(k + 1) * P, 0:h0])
                        nc.sync.dma_start(w12_sb[k][:, 0:NT],
                                          w12rT[k * P:(k + 1) * P, 0:NT])
                        nc.sync.dma_start(xg_sb[k][:, h0:nt0],
                                          xgT[k * P:(k + 1) * P, h0:nt0])
                        nc.sync.dma_start(w12_sb[k][:, NT:HR],
                                          w12rT[k * P:(k + 1) * P, NT:HR])
                    else:
                        nc.sync.dma_start(xg_sb[k][:, 0:nt0],
                                          xgT[k * P:(k + 1) * P, 0:nt0])
                        nc.sync.dma_start(w12_sb[k][:, 0:HR],
                                          w12rT[k * P:(k + 1) * P, 0:HR])
                for k in range(KD):
                    nc.sync.dma_start(w12_sb[k][:, HR:],
                                      w12rT[k * P:(k + 1) * P, HR:])
                for k in range(KH):
                    nc.sync.dma_start(w3_sb[k][:], w3T[k * P:(k + 1) * P, :])
                if C > nt0:
                    for k in range(KD):
                        nc.sync.dma_start(xg_sb[k][:, nt0:],
                                          xgT[k * P:(k + 1) * P, nt0:])

                if early_prefetch:
                    # shared-phase critical prefetch streams behind the
                    # routed loads
                    emit_prefetch_a()

                # routed compute
                off = 0
                for b, nt in enumerate(blk_w):
                    tok = slice(off, off + nt)
                    off += nt
                    hs = []
                    # first chunks are single k-tiles: the first matmuls
                    # gate on one xg+w12 tile (~380KB) instead of four, so
                    # real work starts ~2us sooner while weights stream
                    kchunks = ([range(0, 1), range(1, 2), range(2, 4),
                                range(4, 8), range(8, 12), range(12, 16)]
                               if b == 0 else
                               [range(0, 4), range(4, 8), range(8, 12),
                                range(12, 16)])
                    for mg in range(2):
                        pg = [psR.tile([P, NT], F32, name=f"pb{m}", tag=f"pb{m}")
                              for m in range(4)]
                        pu = [psR.tile([P, NT], F32, name=f"pb{4+m}", tag=f"pb{4+m}")
                              for m in range(4)]
                        # k emitted in chunks so the first matmuls only
                        # depend on the first few weight tiles
                        for ks in kchunks:
                            for m in range(4):
                                # gate / up columns inside the mg half
                                c1 = slice(mg * HR + m * P, mg * HR + (m + 1) * P)
                                c2 = slice(mg * HR + NT + m * P,
                                           mg * HR + NT + (m + 1) * P)
                                for k in ks:
                                    nc.tensor.matmul(pg[m][:, :nt],
                                                     w12_sb[k][:, c1],
                                                     xg_sb[k][:, tok],
                                                     start=(k == 0), stop=(k == KD - 1))
                                for k in ks:
                                    nc.tensor.matmul(pu[m][:, :nt],
                                                     w12_sb[k][:, c2],
                                                     xg_sb[k][:, tok],
                                                     start=(k == 0), stop=(k == KD - 1))
                        for m in range(4):
                            sg = tpool.tile([P, NT], F32, name="sg", tag="sg")
                            nc.scalar.activation(sg[:, :nt], pg[m][:, :nt], AF.Silu)
                            h = hpool.tile([P, NT], BF16, name=f"h_{mg*4+m}",
                                           tag=f"h_{mg*4+m}")
                            nc.vector.tensor_mul(h[:, :nt], sg[:, :nt], pu[m][:, :nt])
                            hs.append(h)
                    last_blk = (b == NBLK - 1)
                    for mo in range(KD):
                        # On the last block, flip the mo->PSUM-tag map: the
                        # shared phase's first psA tiles reallocate the
                        # last-freed banks (observed: the first shared matmul
                        # waited on the FINAL routed copy), so hand those
                        # banks to the earliest-copied mo tiles instead.
                        pb = (7 - mo % 8) if last_blk else (mo % 8)
                        po = psR.tile([P, NT], F32, name=f"pb{pb}", tag=f"pb{pb}")
                        for k in range(KH):
                            nc.tensor.matmul(po[:, :nt],
                                             w3_sb[k][:, mo * P:(mo + 1) * P],
                                             hs[k][:, :nt],
                                             start=(k == 0), stop=(k == KH - 1))
                        so = opool.tile([P, NT], BF16, name="so", tag="so")
                        nc.vector.tensor_copy(so[:, :nt], po[:, :nt])
                        # outputs ride the SWDGE so the sync queue keeps
                        # streaming weights
                        nc.gpsimd.dma_start(routed_outT[mo * P:(mo + 1) * P, tok],
                                            so[:, :nt])

            # ---------------- Phase S: shared expert ----------------
            wsh2 = ctx.enter_context(tc.tile_pool(name="w_sh2", bufs=1))
            wdn = ctx.enter_context(tc.tile_pool(name="w_dn", bufs=1))
            xpool = ctx.enter_context(tc.tile_pool(name="x_p", bufs=2))
            psA = ctx.enter_context(tc.tile_pool(name="psA", bufs=2, space="PSUM"))
            psB = ctx.enter_context(tc.tile_pool(name="psB", bufs=4, space="PSUM"))

            if not early_prefetch:
                emit_prefetch_a()

            # second weight halves, down-proj weights, chunk-0/sb-1 x:
            # these land in SBUF freed by the routed pools (WAR-ordered).
            # sw3 and x01 interleave so neither arrives at the wire.
            sw1_h2 = [wsh2.tile([P, H2], BF16, name=f"sw1_{k}_1", tag=f"sw1_{k}_1")
                      for k in range(KD)]
            sw2_h2 = [wsh2.tile([P, H2], BF16, name=f"sw2_{k}_1", tag=f"sw2_{k}_1")
                      for k in range(KD)]
            sw3_sb = [wdn.tile([P, D], BF16, name=f"sw3_{k}", tag=f"sw3_{k}")
                      for k in range(KH)]
            x01 = [xpool.tile([P, NT], BF16, name=f"x_{k}_1", tag=f"x_{k}_1")
                   for k in range(KD)]
            for k in range(KD):
                nc.sync.dma_start(sw1_h2[k][:], sw1T[k * P:(k + 1) * P, H2:])
                nc.sync.dma_start(sw2_h2[k][:], sw2T[k * P:(k + 1) * P, H2:])
            for k in range(KH // 2):
                nc.sync.dma_start(sw3_sb[k][:], sw3T[k * P:(k + 1) * P, :])
            for k in range(KD // 2):
                nc.sync.dma_start(x01[k][:], xT[k * P:(k + 1) * P, NT:CH])
            for k in range(KH // 2, KH):
                nc.sync.dma_start(sw3_sb[k][:], sw3T[k * P:(k + 1) * P, :])
            for k in range(KD // 2, KD):
                nc.sync.dma_start(x01[k][:], xT[k * P:(k + 1) * P, NT:CH])

            sw_h = [[sw1_h1, sw1_h2], [sw2_h1, sw2_h2]]

            for ch in range(T // CH):
                if ch == 0:
                    x_sb = [[x00[k], x01[k]] for k in range(KD)]
                else:
                    x_sb = [[xpool.tile([P, NT], BF16, name=f"x_{k}_{h}",
                                        tag=f"x_{k}_{h}")
                             for h in range(CH // NT)] for k in range(KD)]
                    for k in range(KD):
                        for h in range(CH // NT):
                            nc.sync.dma_start(
                                x_sb[k][h][:],
                                xT[k * P:(k + 1) * P,
                                   ch * CH + h * NT:ch * CH + (h + 1) * NT])
                for sb in range(CH // NT):
                    otok = slice(ch * CH + sb * NT, ch * CH + (sb + 1) * NT)
                    hs = []
                    for m in range(KH):
                        wh, wm = divmod(m, H2 // P)   # which weight half-tile
                        mm = slice(wm * P, (wm + 1) * P)
                        pg = psA.tile([P, NT], F32, name="pg", tag="pg")
                        pu = psA.tile([P, NT], F32, name="pu", tag="pu")
                        for k in range(KD):
                            nc.tensor.matmul(pg[:], sw_h[0][wh][k][:, mm],
                                             x_sb[k][sb][:],
                                             start=(k == 0), stop=(k == KD - 1))
                        for k in range(KD):
                            nc.tensor.matmul(pu[:], sw_h[1][wh][k][:, mm],
                                             x_sb[k][sb][:],
                                             start=(k == 0), stop=(k == KD - 1))
                        sg = tpool.tile([P, NT], F32, name="sg", tag="sg")
                        nc.scalar.activation(sg[:], pg[:], AF.Silu)
                        h = hpool.tile([P, NT], BF16, name=f"h_{m}", tag=f"h_{m}")
                        nc.vector.tensor_mul(h[:], sg[:], pu[:])
                        hs.append(h)
                    last_sb = (ch == T // CH - 1 and sb == CH // NT - 1)
                    for mo in range(KD):
                        orow = slice(mo * P, (mo + 1) * P)
                        if last_sb and mo >= KD - 2:
                            # final two tiles in half-width groups (separate
                            # PSUM banks), copies rotated across engines and
                            # DMAs split over both idle HWDGE queues, so the
                            # post-matmul drain is one [128,256] copy + one
                            # cheap SWDGE issue
                            dma_eng = [nc.sync, nc.gpsimd, nc.sync, nc.gpsimd]
                            for hf in range(2):
                                pi = (mo - (KD - 2)) * 2 + hf
                                cs = slice(hf * (NT // 2), (hf + 1) * (NT // 2))
                                po = psB.tile([P, NT], F32, name="po", tag="po")
                                for k in range(KH):
                                    nc.tensor.matmul(po[:, :NT // 2],
                                                     sw3_sb[k][:, orow],
                                                     hs[k][:, cs],
                                                     start=(k == 0), stop=(k == KH - 1))
                                so = opool.tile([P, NT], BF16, name="so", tag="so")
                                nc.vector.tensor_copy(so[:, :NT // 2],
                                                      po[:, :NT // 2])
                                dma_eng[pi].dma_start(
                                    shared_outT[orow,
                                                otok.start + hf * (NT // 2):
                                                otok.start + (hf + 1) * (NT // 2)],
                                    so[:, :NT // 2])
                            continue
                        po = psB.tile([P, NT], F32, name="po", tag="po")
                        for k in range(KH):
                            nc.tensor.matmul(po[:], sw3_sb[k][:, orow],
                                             hs[k][:],
                                             start=(k == 0), stop=(k == KH - 1))
                        so = opool.tile([P, NT], BF16, name="so", tag="so")
                        nc.vector.tensor_copy(so[:], po[:])
                        if last_sb:
                            # loads are done; split the drain DMAs across the
                            # idle HWDGE and the SWDGE
                            (nc.sync if mo % 2 == 0 else nc.gpsimd).dma_start(
                                shared_outT[orow, otok], so[:])
                        else:
                            nc.gpsimd.dma_start(shared_outT[orow, otok], so[:])

    nc.compile()
    return nc


_PROGRAM_CACHE: dict = {}


def _get_program(C: int):
    if C not in _PROGRAM_CACHE:
        _PROGRAM_CACHE[C] = _build_program(C)
    return _PROGRAM_CACHE[C]


def _route_like_reference(xf: np.ndarray, router_w: np.ndarray,
                          expert_bias: np.ndarray):
    """Router computed with jax on CPU to bit-match the reference's top-k."""
    import jax
    import jax.numpy as jnp

    cpu = jax.devices("cpu")[0]
    with jax.default_device(cpu):
        xj = jnp.asarray(xf)
        scores = jax.nn.sigmoid(xj @ jnp.asarray(router_w).T)        # (T, E)
        sel = scores + jnp.asarray(expert_bias)
        _, top_idx = jax.lax.top_k(sel, TOPK)                        # (T, K)
        top_sc = jnp.take_along_axis(scores, top_idx, axis=-1)
        top_w = top_sc / (top_sc.sum(-1, keepdims=True) + 1e-9)
        return np.asarray(top_idx), np.asarray(top_w)


def kernel(x, w12, w3, router_w, expert_bias, sw1, sw2, sw3):
    x = np.asarray(x, dtype=np.float32)
    w12 = np.asarray(w12, dtype=np.float32)
    w3 = np.asarray(w3, dtype=np.float32)
    router_w = np.asarray(router_w, dtype=np.float32)
    expert_bias = np.asarray(expert_bias, dtype=np.float32)
    sw1 = np.asarray(sw1, dtype=np.float32)
    sw2 = np.asarray(sw2, dtype=np.float32)
    sw3 = np.asarray(sw3, dtype=np.float32)

    xf = x.reshape(T, D)
    top_idx, top_w = _route_like_reference(xf, router_w, expert_bias)

    # per-expert token lists + combine weights
    idx_list, w_list = [], []
    for e in range(E):
        hit = top_idx == e                      # (T, K)
        tok = np.nonzero(hit.any(axis=1))[0]
        wt = (top_w * hit).sum(axis=1)[tok]     # combine weight per token
        idx_list.append(tok.astype(np.int64))
        w_list.append(wt.astype(np.float32))

    max_n = max(len(i) for i in idx_list)
    # Device capacity policy: cap at C_CORE (the exact mean load for top-2 of
    # 8 experts) and fix up small per-expert overflows on host in fp32
    # (<0.2% of FLOPs, like the router). Grossly imbalanced routing falls
    # back to extra device launches in slabs of C_MAX.
    C_CORE = 1024
    C_MAX = 1280   # slab size for the imbalanced-routing fallback (SBUF limit)
    overflow = sum(max(0, len(i) - C_CORE) for i in idx_list)
    if max_n <= C_CORE:
        C = max(P, -(-max_n // P) * P)          # capacity, multiple of 128
        n_launches, host_fix = 1, False
    elif overflow <= 1024:
        C, n_launches, host_fix = C_CORE, 1, True
    else:
        C = C_MAX
        n_launches, host_fix = max(1, -(-max_n // C_MAX)), False

    xT16 = np.ascontiguousarray(xf.T).astype(ml_dtypes.bfloat16)   # (D, T)

    nc = _get_program(C)

    sw_z = np.zeros((D, HC), dtype=ml_dtypes.bfloat16)
    sw3_z = np.zeros((HC, D), dtype=ml_dtypes.bfloat16)

    outT = np.zeros((D, T), dtype=np.float32)
    global _LAST_RESULTS
    for launch in range(n_launches):
        lo = launch * C_MAX
        in_maps = []
        for c in range(NCORES):
            hs = slice(c * HC, (c + 1) * HC)
            idx_c = idx_list[c][lo:lo + C]
            w_c = w_list[c][lo:lo + C]
            n_c = len(idx_c)
            xg = np.zeros((D, C), dtype=ml_dtypes.bfloat16)
            xg[:, :n_c] = xT16[:, idx_c]
            if launch == 0:
                s1 = np.ascontiguousarray(sw1[hs].T).astype(ml_dtypes.bfloat16)
                s2 = np.ascontiguousarray(sw2[hs].T).astype(ml_dtypes.bfloat16)
                s3 = np.ascontiguousarray(sw3[:, hs].T).astype(ml_dtypes.bfloat16)
            else:
                s1, s2, s3 = sw_z, sw_z, sw3_z   # shared part already done
            # reorder w12 columns into [gate m0-3 | up m0-3 | gate m4-7 | up m4-7]
            w12t = np.ascontiguousarray(w12[c].T).astype(ml_dtypes.bfloat16)
            w12r = np.concatenate([w12t[:, 0:NT], w12t[:, HR:HR + NT],
                                   w12t[:, NT:HR], w12t[:, HR + NT:]], axis=1)
            in_maps.append({
                "xT": xT16,
                "sw1T": s1, "sw2T": s2, "sw3T": s3,
                "w12rT": np.ascontiguousarray(w12r),
                "w3T": np.ascontiguousarray(w3[c].T).astype(ml_dtypes.bfloat16),
                "xgT": xg,
            })

        res = run_bass_kernel_spmd(nc, in_maps, core_ids=list(range(NCORES)),
                                   **_RUN_KWARGS)
        _LAST_RESULTS = res

        for c in range(NCORES):
            if launch == 0:
                outT += res.results[c]["shared_outT"].astype(np.float32)
            idx_c = idx_list[c][lo:lo + C]
            if len(idx_c):
                # token indices are unique within one expert; combine weight
                # applied here in fp32
                ro = res.results[c]["routed_outT"][:, :len(idx_c)].astype(np.float32)
                outT[:, idx_c] += ro * w_list[c][lo:lo + C][None, :]

    if host_fix:
        # fp32 fixup for tokens beyond the device capacity of each expert
        for c in range(NCORES):
            tail = idx_list[c][C:]
            if len(tail) == 0:
                continue
            wts = w_list[c][C:]
            xs = xf[tail]                             # (n, D)
            h12 = xs @ w12[c].T                       # (n, 2*HR)
            h1, h2 = h12[:, :HR], h12[:, HR:]
            h = h1 / (1.0 + np.exp(-h1)) * h2         # silu(h1) * h2
            out = (h * wts[:, None]) @ w3[c].T        # (n, D)
            outT[:, tail] += out.T
    return outT.T.reshape(B, S, D).astype(np.float32)


# test harness hooks: set _RUN_KWARGS = {"trace": True, ...} before calling
# kernel() to profile; read _LAST_RESULTS afterwards.
_RUN_KWARGS: dict = {}
_LAST_RESULTS = None



# revision 19
# speedup vs baseline: 1.0152x; 1.0039x over previous
"""MoE (shared expert + 8 routed experts, top-2) on 8 Trainium2 NeuronCores.

Sharding: core c holds
  - shared-expert slice c: rows [c*1024, (c+1)*1024) of sw1/sw2 and the
    matching columns of sw3  -> partial (T, D) output, summed on host
  - routed expert c's weights (w12[c], w3[c]); host routes/gathers the
    tokens selected for expert c (capacity 1024 = the exact mean load),
    device computes unscaled expert outputs, host applies combine weights
    during the fp32 scatter-add; small per-expert overflows beyond the
    capacity are fixed up on host in fp32.

Device math is bf16 with fp32 PSUM accumulation; outputs are written
bf16 and promoted to fp32 during the host-side reduce.

v2 schedule: the routed expert runs FIRST (its 16.8MB of weights stream
at t=0 when the DMA queue is otherwise idle, with the contraction loop
ordered so matmuls start after ~0.5MB has landed); the shared-expert
weights prefetch behind routed compute so the phase transition has no
DMA bubble.  The v1 schedule ran shared-first and paid a ~30us stall +
HAM cold-clock window when the routed weights all loaded at the end.
"""

import sys

if "/opt/trn_rl_repo" not in sys.path:
    sys.path.insert(0, "/opt/trn_rl_repo")

from contextlib import ExitStack

import numpy as np
import ml_dtypes

import concourse.bass as bass
import concourse.tile as tile
from concourse import mybir, bacc
from concourse.bass_utils import run_bass_kernel_spmd

BF16 = mybir.dt.bfloat16
F32 = mybir.dt.float32
AF = mybir.ActivationFunctionType

# Problem shape (hardcoded per spec)
B, S, D = 2, 2048, 2048
T = B * S                  # 4096 tokens
E = 8                      # routed experts == n_cores
TOPK = 2
H_SHARED = 8192
HC = H_SHARED // 8         # shared-expert hidden slice per core
HR = 1024                  # routed expert hidden
NCORES = 8
NT = 512                   # token block (one PSUM bank at fp32)
P = 128
CH = 1024                  # shared-phase x chunk (2 sub-blocks)


def _build_program(C: int):
    """SPMD Bass program, routed capacity C (multiple of 128).

    Routed phase first, then shared; shared weights prefetch during the
    routed phase when C <= 1024 (SBUF budget), else after it.
    """
    nc = bacc.Bacc("TRN2", target_bir_lowering=False, debug=False)

    xT = nc.dram_tensor("xT", [D, T], BF16, kind="ExternalInput")
    sw1T = nc.dram_tensor("sw1T", [D, HC], BF16, kind="ExternalInput")
    sw2T = nc.dram_tensor("sw2T", [D, HC], BF16, kind="ExternalInput")
    sw3T = nc.dram_tensor("sw3T", [HC, D], BF16, kind="ExternalInput")
    # w12rT columns: [gate m0-3 | up m0-3 | gate m4-7 | up m4-7]
    w12rT = nc.dram_tensor("w12rT", [D, 2 * HR], BF16, kind="ExternalInput")
    w3T = nc.dram_tensor("w3T", [HR, D], BF16, kind="ExternalInput")
    xgT = nc.dram_tensor("xgT", [D, C], BF16, kind="ExternalInput")

    shared_outT = nc.dram_tensor("shared_outT", [D, T], BF16, kind="ExternalOutput")
    routed_outT = nc.dram_tensor("routed_outT", [D, C], BF16, kind="ExternalOutput")

    KD = D // P    # 16 contraction tiles over D
    KH = HC // P   # 8 contraction tiles over HC (== HR // P)
    H2 = HC // 2   # shared up-proj weight half-tile width

    # routed token blocks
    blk_w = [NT] * (C // NT)
    if C % NT:
        blk_w.append(C % NT)
    NBLK = len(blk_w)
    early_prefetch = C <= 1024

    with tile.TileContext(nc) as tc:
        with ExitStack() as ctx:
            # pools that live across both phases
            hpool = ctx.enter_context(tc.tile_pool(name="h_p", bufs=2))
            tpool = ctx.enter_context(tc.tile_pool(name="t_p", bufs=2))
            opool = ctx.enter_context(
                tc.tile_pool(name="o_p", bufs=4 if C <= 1024 else 2))
            # shared-phase first-half weights + chunk-0/sb-0 x: entered
            # before the routed pools (pool release is LIFO) so they
            # survive into the shared phase
            wsh1 = ctx.enter_context(tc.tile_pool(name="w_sh1", bufs=1))
            xA0 = ctx.enter_context(tc.tile_pool(name="x_a0", bufs=1))
            sw1_h1 = [wsh1.tile([P, H2], BF16, name=f"sw1_{k}_0",
                                tag=f"sw1_{k}_0") for k in range(KD)]
            sw2_h1 = [wsh1.tile([P, H2], BF16, name=f"sw2_{k}_0",
                                tag=f"sw2_{k}_0") for k in range(KD)]
            x00 = [xA0.tile([P, NT], BF16, name=f"xa_{k}", tag=f"xa_{k}")
                   for k in range(KD)]

            # input DMAs stay on the sync queue alone: the stream is DMA-ring
            # completion-limited (~wire speed), so a second issue queue only
            # reorders arrivals, and the scalar queue must stay clear for the
            # silu chain (a backlog there stalls the PE on PSUM WAR)
            def emit_prefetch_a():
                for k in range(KD):
                    nc.sync.dma_start(sw1_h1[k][:], sw1T[k * P:(k + 1) * P, :H2])
                    nc.sync.dma_start(sw2_h1[k][:], sw2T[k * P:(k + 1) * P, :H2])
                for k in range(KD):
                    nc.sync.dma_start(x00[k][:], xT[k * P:(k + 1) * P, 0:NT])

            # HAM warmup: dummy matmuls on uninitialized SBUF (no deps, so
            # they issue right after the engine preamble) keep the PE busy
            # through its cold-clock window while the first weights stream
            # in; without them the first ~3.4us of real matmuls run at
            # half clock. Results land in a scratch PSUM bank, never read.
            wmp = ctx.enter_context(tc.tile_pool(name="wm_p", bufs=1))
            wwm = wmp.tile([P, P], BF16, name="wwm", tag="wwm")
            xwm = wmp.tile([P, NT // 2], BF16, name="xwm", tag="xwm")
            nc.vector.memset(wwm[:], 0)
            nc.gpsimd.memset(xwm[:], 0)

            with ExitStack() as ctx_r:
                # ---------------- Phase R: routed expert ----------------
                wr = ctx_r.enter_context(tc.tile_pool(name="w_r", bufs=1))
                xgp = ctx_r.enter_context(tc.tile_pool(name="xg_p", bufs=1))
                psR = ctx_r.enter_context(
                    tc.tile_pool(name="psR", bufs=1, space="PSUM"))

                for g in range(2):
                    pw = psR.tile([P, NT], F32, name="pb7", tag="pb7")
                    for k in range(5):
                        nc.tensor.matmul(pw[:, :NT // 2], wwm[:], xwm[:],
                                         start=(k == 0), stop=(k == 4))

                # full-width tiles, one DMA each: the sync engine issues
                # DMA instructions serially (~0.6us apiece, ring depth 1),
                # so fewer/bigger transfers beat many small ones
                w12_sb = [wr.tile([P, 2 * HR], BF16, name=f"w12_{k}",
                                  tag=f"w12_{k}") for k in range(KD)]
                w3_sb = [wr.tile([P, D], BF16, name=f"w3_{k}", tag=f"w3_{k}")
                         for k in range(KH)]
                xg_sb = [xgp.tile([P, C], BF16, name=f"xg_{k}", tag=f"xg_{k}")
                         for k in range(KD)]

                # DMA emission = execution order on the sync queue,
                # need-ordered: block-0 tokens + mg0 weight halves feed the
                # first matmul wave with exactly its bytes; then mg1
                # halves, then w3 (block-0 down-proj), then the remaining
                # tokens (block 1+)
                nt0 = min(NT, C)
                for k in range(KD):
                    if k < 2:
                        # first tiles split small across DMA rings: the
                        # first matmuls' dependencies land ~2us sooner
                        h0 = nt0 // 2
                        nc.sync.dma_start(xg_sb[k][:, 0:h0],
                                          xgT[k * P:(k + 1) * P, 0:h0])
                        nc.sync.dma_start(w12_sb[k][:, 0:NT],
                                          w12rT[k * P:(k + 1) * P, 0:NT])
                        nc.sync.dma_start(xg_sb[k][:, h0:nt0],
                                          xgT[k * P:(k + 1) * P, h0:nt0])
                        nc.sync.dma_start(w12_sb[k][:, NT:HR],
                                          w12rT[k * P:(k + 1) * P, NT:HR])
                    else:
                        nc.sync.dma_start(xg_sb[k][:, 0:nt0],
                                          xgT[k * P:(k + 1) * P, 0:nt0])
                        nc.sync.dma_start(w12_sb[k][:, 0:HR],
                                          w12rT[k * P:(k + 1) * P, 0:HR])
                for k in range(KD):
                    nc.sync.dma_start(w12_sb[k][:, HR:],
                                      w12rT[k * P:(k + 1) * P, HR:])
                for k in range(KH):
                    nc.sync.dma_start(w3_sb[k][:], w3T[k * P:(k + 1) * P, :])
                if C > nt0:
                    for k in range(KD):
                        nc.sync.dma_start(xg_sb[k][:, nt0:],
                                          xgT[k * P:(k + 1) * P, nt0:])

                if early_prefetch:
                    # shared-phase critical prefetch streams behind the
                    # routed loads
                    emit_prefetch_a()

                # routed compute
                off = 0
                for b, nt in enumerate(blk_w):
                    tok = slice(off, off + nt)
                    off += nt
                    hs = []
                    # first chunks are single k-tiles: the first matmuls
                    # gate on one xg+w12 tile (~380KB) instead of four, so
                    # real work starts ~2us sooner while weights stream
                    kchunks = ([range(0, 1), range(1, 2), range(2, 4),
                                range(4, 8), range(8, 12), range(12, 16)]
                               if b == 0 else
                               [range(0, 4), range(4, 8), range(8, 12),
                                range(12, 16)])
                    for mg in range(2):
                        pg = [psR.tile([P, NT], F32, name=f"pb{m}", tag=f"pb{m}")
                              for m in range(4)]
                        pu = [psR.tile([P, NT], F32, name=f"pb{4+m}", tag=f"pb{4+m}")
                              for m in range(4)]
                        # k emitted in chunks so the first matmuls only
                        # depend on the first few weight tiles
                        for ks in kchunks:
                            for m in range(4):
                                # gate / up columns inside the mg half
                                c1 = slice(mg * HR + m * P, mg * HR + (m + 1) * P)
                                c2 = slice(mg * HR + NT + m * P,
                                           mg * HR + NT + (m + 1) * P)
                                for k in ks:
                                    nc.tensor.matmul(pg[m][:, :nt],
                                                     w12_sb[k][:, c1],
                                                     xg_sb[k][:, tok],
                                                     start=(k == 0), stop=(k == KD - 1))
                                for k in ks:
                                    nc.tensor.matmul(pu[m][:, :nt],
                                                     w12_sb[k][:, c2],
                                                     xg_sb[k][:, tok],
                                                     start=(k == 0), stop=(k == KD - 1))
                        for m in range(4):
                            sg = tpool.tile([P, NT], F32, name="sg", tag="sg")
                            nc.scalar.activation(sg[:, :nt], pg[m][:, :nt], AF.Silu)
                            h = hpool.tile([P, NT], BF16, name=f"h_{mg*4+m}",
                                           tag=f"h_{mg*4+m}")
                            nc.vector.tensor_mul(h[:, :nt], sg[:, :nt], pu[m][:, :nt])
                            hs.append(h)
                    last_blk = (b == NBLK - 1)
                    for mo in range(KD):
                        # On the last block, flip the mo->PSUM-tag map: the
                        # shared phase's first psA tiles reallocate the
                        # last-freed banks (observed: the first shared matmul
                        # waited on the FINAL routed copy), so hand those
                        # banks to the earliest-copied mo tiles instead.
                        pb = (7 - mo % 8) if last_blk else (mo % 8)
                        po = psR.tile([P, NT], F32, name=f"pb{pb}", tag=f"pb{pb}")
                        for k in range(KH):
                            nc.tensor.matmul(po[:, :nt],
                                             w3_sb[k][:, mo * P:(mo + 1) * P],
                                             hs[k][:, :nt],
                                             start=(k == 0), stop=(k == KH - 1))
                        so = opool.tile([P, NT], BF16, name="so", tag="so")
                        nc.vector.tensor_copy(so[:, :nt], po[:, :nt])
                        # outputs ride the SWDGE so the sync queue keeps
                        # streaming weights
                        nc.gpsimd.dma_start(routed_outT[mo * P:(mo + 1) * P, tok],
                                            so[:, :nt])

            # ---------------- Phase S: shared expert ----------------
            wsh2 = ctx.enter_context(tc.tile_pool(name="w_sh2", bufs=1))
            wdn = ctx.enter_context(tc.tile_pool(name="w_dn", bufs=1))
            xpool = ctx.enter_context(tc.tile_pool(name="x_p", bufs=2))
            psA = ctx.enter_context(tc.tile_pool(name="psA", bufs=2, space="PSUM"))
            psB = ctx.enter_context(tc.tile_pool(name="psB", bufs=4, space="PSUM"))

            if not early_prefetch:
                emit_prefetch_a()

            # second weight halves, down-proj weights, chunk-0/sb-1 x:
            # these land in SBUF freed by the routed pools (WAR-ordered).
            # sw3 and x01 interleave so neither arrives at the wire.
            sw1_h2 = [wsh2.tile([P, H2], BF16, name=f"sw1_{k}_1", tag=f"sw1_{k}_1")
                      for k in range(KD)]
            sw2_h2 = [wsh2.tile([P, H2], BF16, name=f"sw2_{k}_1", tag=f"sw2_{k}_1")
                      for k in range(KD)]
            sw3_sb = [wdn.tile([P, D], BF16, name=f"sw3_{k}", tag=f"sw3_{k}")
                      for k in range(KH)]
            x01 = [xpool.tile([P, NT], BF16, name=f"x_{k}_1", tag=f"x_{k}_1")
                   for k in range(KD)]
            for k in range(KD):
                nc.sync.dma_start(sw1_h2[k][:], sw1T[k * P:(k + 1) * P, H2:])
                nc.sync.dma_start(sw2_h2[k][:], sw2T[k * P:(k + 1) * P, H2:])
            for k in range(KH // 2):
                nc.sync.dma_start(sw3_sb[k][:], sw3T[k * P:(k + 1) * P, :])
            for k in range(KD // 2):
                nc.sync.dma_start(x01[k][:], xT[k * P:(k + 1) * P, NT:CH])
            for k in range(KH // 2, KH):
                nc.sync.dma_start(sw3_sb[k][:], sw3T[k * P:(k + 1) * P, :])
            for k in range(KD // 2, KD):
                nc.sync.dma_start(x01[k][:], xT[k * P:(k + 1) * P, NT:CH])

            sw_h = [[sw1_h1, sw1_h2], [sw2_h1, sw2_h2]]

            for ch in range(T // CH):
                if ch == 0:
                    x_sb = [[x00[k], x01[k]] for k in range(KD)]
                else:
                    x_sb = [[xpool.tile([P, NT], BF16, name=f"x_{k}_{h}",
                                        tag=f"x_{k}_{h}")
                             for h in range(CH // NT)] for k in range(KD)]
                    for k in range(KD):
                        for h in range(CH // NT):
                            nc.sync.dma_start(
                                x_sb[k][h][:],
                                xT[k * P:(k + 1) * P,
                                   ch * CH + h * NT:ch * CH + (h + 1) * NT])
                for sb in range(CH // NT):
                    otok = slice(ch * CH + sb * NT, ch * CH + (sb + 1) * NT)
                    hs = []
                    for m in range(KH):
                        wh, wm = divmod(m, H2 // P)   # which weight half-tile
                        mm = slice(wm * P, (wm + 1) * P)
                        pg = psA.tile([P, NT], F32, name="pg", tag="pg")
                        pu = psA.tile([P, NT], F32, name="pu", tag="pu")
                        for k in range(KD):
                            nc.tensor.matmul(pg[:], sw_h[0][wh][k][:, mm],
                                             x_sb[k][sb][:],
                                             start=(k == 0), stop=(k == KD - 1))
                        for k in range(KD):
                            nc.tensor.matmul(pu[:], sw_h[1][wh][k][:, mm],
                                             x_sb[k][sb][:],
                                             start=(k == 0), stop=(k == KD - 1))
                        sg = tpool.tile([P, NT], F32, name="sg", tag="sg")
                        nc.scalar.activation(sg[:], pg[:], AF.Silu)
                        h = hpool.tile([P, NT], BF16, name=f"h_{m}", tag=f"h_{m}")
                        nc.vector.tensor_mul(h[:], sg[:], pu[:])
                        hs.append(h)
                    last_sb = (ch == T // CH - 1 and sb == CH // NT - 1)
                    for mo in range(KD):
                        orow = slice(mo * P, (mo + 1) * P)
                        if last_sb and mo >= KD - 2:
                            # final two tiles in half-width groups (separate
                            # PSUM banks), copies rotated across engines and
                            # DMAs split over both idle HWDGE queues, so the
                            # post-matmul drain is one [128,256] copy + one
                            # cheap SWDGE issue
                            dma_eng = [nc.sync, nc.scalar, nc.sync, nc.scalar]
                            for hf in range(2):
                                pi = (mo - (KD - 2)) * 2 + hf
                                cs = slice(hf * (NT // 2), (hf + 1) * (NT // 2))
                                po = psB.tile([P, NT], F32, name="po", tag="po")
                                for k in range(KH):
                                    nc.tensor.matmul(po[:, :NT // 2],
                                                     sw3_sb[k][:, orow],
                                                     hs[k][:, cs],
                                                     start=(k == 0), stop=(k == KH - 1))
                                so = opool.tile([P, NT], BF16, name="so", tag="so")
                                nc.vector.tensor_copy(so[:, :NT // 2],
                                                      po[:, :NT // 2])
                                dma_eng[pi].dma_start(
                                    shared_outT[orow,
                                                otok.start + hf * (NT // 2):
                                                otok.start + (hf + 1) * (NT // 2)],
                                    so[:, :NT // 2])
                            continue
                        po = psB.tile([P, NT], F32, name="po", tag="po")
                        for k in range(KH):
                            nc.tensor.matmul(po[:], sw3_sb[k][:, orow],
                                             hs[k][:],
                                             start=(k == 0), stop=(k == KH - 1))
                        so = opool.tile([P, NT], BF16, name="so", tag="so")
                        nc.vector.tensor_copy(so[:], po[:])
                        if ch == T // CH - 1:
                            # last chunk: outputs ride the two fast HWDGE
                            # queues (both idle by now) so the slow SWDGE is
                            # fully drained well before the end-of-kernel
                            # DMA drain barrier
                            (nc.sync if mo % 2 == 0 else nc.scalar).dma_start(
                                shared_outT[orow, otok], so[:])
                        else:
                            nc.gpsimd.dma_start(shared_outT[orow, otok], so[:])

    nc.compile()
    return nc


_PROGRAM_CACHE: dict = {}


def _get_program(C: int):
    if C not in _PROGRAM_CACHE:
        _PROGRAM_CACHE[C] = _build_program(C)
    return _PROGRAM_CACHE[C]


def _route_like_reference(xf: np.ndarray, router_w: np.ndarray,
                          expert_bias: np.ndarray):
    """Router computed with jax on CPU to bit-match the reference's top-k."""
    import jax
    import jax.numpy as jnp

    cpu = jax.devices("cpu")[0]
    with jax.default_device(cpu):
        xj = jnp.asarray(xf)
        scores = jax.nn.sigmoid(xj @ jnp.asarray(router_w).T)        # (T, E)
        sel = scores + jnp.asarray(expert_bias)
        _, top_idx = jax.lax.top_k(sel, TOPK)                        # (T, K)
        top_sc = jnp.take_along_axis(scores, top_idx, axis=-1)
        top_w = top_sc / (top_sc.sum(-1, keepdims=True) + 1e-9)
        return np.asarray(top_idx), np.asarray(top_w)


def kernel(x, w12, w3, router_w, expert_bias, sw1, sw2, sw3):
    x = np.asarray(x, dtype=np.float32)
    w12 = np.asarray(w12, dtype=np.float32)
    w3 = np.asarray(w3, dtype=np.float32)
    router_w = np.asarray(router_w, dtype=np.float32)
    expert_bias = np.asarray(expert_bias, dtype=np.float32)
    sw1 = np.asarray(sw1, dtype=np.float32)
    sw2 = np.asarray(sw2, dtype=np.float32)
    sw3 = np.asarray(sw3, dtype=np.float32)

    xf = x.reshape(T, D)
    top_idx, top_w = _route_like_reference(xf, router_w, expert_bias)

    # per-expert token lists + combine weights
    idx_list, w_list = [], []
    for e in range(E):
        hit = top_idx == e                      # (T, K)
        tok = np.nonzero(hit.any(axis=1))[0]
        wt = (top_w * hit).sum(axis=1)[tok]     # combine weight per token
        idx_list.append(tok.astype(np.int64))
        w_list.append(wt.astype(np.float32))

    max_n = max(len(i) for i in idx_list)
    # Device capacity policy: cap at C_CORE (the exact mean load for top-2 of
    # 8 experts) and fix up small per-expert overflows on host in fp32
    # (<0.2% of FLOPs, like the router). Grossly imbalanced routing falls
    # back to extra device launches in slabs of C_MAX.
    C_CORE = 1024
    C_MAX = 1280   # slab size for the imbalanced-routing fallback (SBUF limit)
    overflow = sum(max(0, len(i) - C_CORE) for i in idx_list)
    if max_n <= C_CORE:
        C = max(P, -(-max_n // P) * P)          # capacity, multiple of 128
        n_launches, host_fix = 1, False
    elif overflow <= 1024:
        C, n_launches, host_fix = C_CORE, 1, True
    else:
        C = C_MAX
        n_launches, host_fix = max(1, -(-max_n // C_MAX)), False

    xT16 = np.ascontiguousarray(xf.T).astype(ml_dtypes.bfloat16)   # (D, T)

    nc = _get_program(C)

    sw_z = np.zeros((D, HC), dtype=ml_dtypes.bfloat16)
    sw3_z = np.zeros((HC, D), dtype=ml_dtypes.bfloat16)

    outT = np.zeros((D, T), dtype=np.float32)
    global _LAST_RESULTS
    for launch in range(n_launches):
        lo = launch * C_MAX
        in_maps = []
        for c in range(NCORES):
            hs = slice(c * HC, (c + 1) * HC)
            idx_c = idx_list[c][lo:lo + C]
            w_c = w_list[c][lo:lo + C]
            n_c = len(idx_c)
            xg = np.zeros((D, C), dtype=ml_dtypes.bfloat16)
            xg[:, :n_c] = xT16[:, idx_c]
            if launch == 0:
                s1 = np.ascontiguousarray(sw1[hs].T).astype(ml_dtypes.bfloat16)
                s2 = np.ascontiguousarray(sw2[hs].T).astype(ml_dtypes.bfloat16)
                s3 = np.ascontiguousarray(sw3[:, hs].T).astype(ml_dtypes.bfloat16)
            else:
                s1, s2, s3 = sw_z, sw_z, sw3_z   # shared part already done
            # reorder w12 columns into [gate m0-3 | up m0-3 | gate m4-7 | up m4-7]
            w12t = np.ascontiguousarray(w12[c].T).astype(ml_dtypes.bfloat16)
            w12r = np.concatenate([w12t[:, 0:NT], w12t[:, HR:HR + NT],
                                   w12t[:, NT:HR], w12t[:, HR + NT:]], axis=1)
            in_maps.append({
                "xT": xT16,
                "sw1T": s1, "sw2T": s2, "sw3T": s3,
                "w12rT": np.ascontiguousarray(w12r),
                "w3T": np.ascontiguousarray(w3[c].T).astype(ml_dtypes.bfloat16),
                "xgT": xg,
            })

        res = run_bass_kernel_spmd(nc, in_maps, core_ids=list(range(NCORES)),
                                   **_RUN_KWARGS)
        _LAST_RESULTS = res

        for c in range(NCORES):
            if launch == 0:
                outT += res.results[c]["shared_outT"].astype(np.float32)
            idx_c = idx_list[c][lo:lo + C]
            if len(idx_c):
                # token indices are unique within one expert; combine weight
                # applied here in fp32
                ro = res.results[c]["routed_outT"][:, :len(idx_c)].astype(np.float32)
                outT[:, idx_c] += ro * w_list[c][lo:lo + C][None, :]

    if host_fix:
        # fp32 fixup for tokens beyond the device capacity of each expert
        for c in range(NCORES):
            tail = idx_list[c][C:]
            if len(tail) == 0:
                continue
            wts = w_list[c][C:]
            xs = xf[tail]                             # (n, D)
            h12 = xs @ w12[c].T                       # (n, 2*HR)
            h1, h2 = h12[:, :HR], h12[:, HR:]
            h = h1 / (1.0 + np.exp(-h1)) * h2         # silu(h1) * h2
            out = (h * wts[:, None]) @ w3[c].T        # (n, D)
            outT[:, tail] += out.T
    return outT.T.reshape(B, S, D).astype(np.float32)


# test harness hooks: set _RUN_KWARGS = {"trace": True, ...} before calling
# kernel() to profile; read _LAST_RESULTS afterwards.
_RUN_KWARGS: dict = {}
_LAST_RESULTS = None



# revision 28
# speedup vs baseline: 1.1084x; 1.0918x over previous
"""MoE (shared expert + 8 routed experts, top-2) on 8 Trainium2 NeuronCores.

Sharding: core c holds
  - shared-expert slice c: rows [c*1024, (c+1)*1024) of sw1/sw2 and the
    matching columns of sw3  -> partial (T, D) output, summed on host
  - routed expert c's weights (w12[c], w3[c]); host routes/gathers the
    tokens selected for expert c (capacity 1024 = the exact mean load),
    device computes unscaled expert outputs, host applies combine weights
    during the fp32 scatter-add; small per-expert overflows beyond the
    capacity are fixed up on host in fp32.

Precision: the shared expert runs bf16 with fp32 PSUM (its 8-way
H-sharded contraction sums across cores, so even one fp8 DoubleRow pair
per slice puts 1/4 of the global contraction in fp8 — too noisy).  The
ROUTED expert runs entirely in fp8-e4m3 DoubleRow (2x PE throughput):
its per-expert contraction lives on one core, and full-fp8 routed costs
~1.0e-2 of the 2e-2 error budget.  Scale products are exactly 1
(x*0.25 @ w12*4 for gates; x*0.25 @ w12*2 -> u*0.5 -> h*0.5 fp8 from
the DVE mul; h*0.5 @ w3*2), so fp8 partial sums are true-scale and no
dequant exists anywhere.  Simulated end-to-end max-rel 1.62e-2.

Schedule: routed expert first (fp8 weights stream at t=0, phase is now
~2x shorter), shared-expert weights prefetch behind routed compute;
drain copies/DMAs spread across engines at phase boundaries and the
tail; routed outputs split SWDGE/scalar-HWDGE (the fp8 down-proj
outpaces the SWDGE alone).
"""

import sys

if "/opt/trn_rl_repo" not in sys.path:
    sys.path.insert(0, "/opt/trn_rl_repo")

from contextlib import ExitStack

import numpy as np
import ml_dtypes

import concourse.bass as bass
import concourse.tile as tile
from concourse import mybir, bacc
from concourse.bass_utils import run_bass_kernel_spmd

BF16 = mybir.dt.bfloat16
F32 = mybir.dt.float32
F8 = mybir.dt.float8e4
AF = mybir.ActivationFunctionType
DR = mybir.MatmulPerfMode.DoubleRow
E4M3 = ml_dtypes.float8_e4m3

# Problem shape (hardcoded per spec)
B, S, D = 2, 2048, 2048
T = B * S                  # 4096 tokens
E = 8                      # routed experts == n_cores
TOPK = 2
H_SHARED = 8192
HC = H_SHARED // 8         # shared-expert hidden slice per core
HR = 1024                  # routed expert hidden
NCORES = 8
NT = 512                   # token block (one PSUM bank at fp32)
P = 128
CH = 1024                  # shared-phase x chunk (2 sub-blocks)

KD = D // P                # 16 contraction tiles over D
KH = HC // P               # 8 contraction tiles over HC (== HR // P)
NP_U = 8                   # routed up-proj fp8 pairs (all of D)
NP_D = 4                   # routed down-proj fp8 pairs (all of HR)
H2 = HC // 2               # shared up-proj weight half-tile width


def _build_program(C: int):
    """SPMD Bass program, routed capacity C (multiple of 128)."""
    nc = bacc.Bacc("TRN2", target_bir_lowering=False, debug=False)

    xT = nc.dram_tensor("xT", [D, T], BF16, kind="ExternalInput")
    sw1T = nc.dram_tensor("sw1T", [D, HC], BF16, kind="ExternalInput")
    sw2T = nc.dram_tensor("sw2T", [D, HC], BF16, kind="ExternalInput")
    sw3T = nc.dram_tensor("sw3T", [HC, D], BF16, kind="ExternalInput")
    # fp8 routed tensors in DoubleRow pair layout: row j*128+p holds
    # [chunk(2j) | chunk(2j+1)] planes side by side.
    # w12q columns: [gate m0-3 | up m0-3 | gate m4-7 | up m4-7],
    # gate values w*4, up values w*2 (folds the h*0.5 scale)
    w12qT = nc.dram_tensor("w12qT", [NP_U * P, 2 * 2 * HR], F8, kind="ExternalInput")
    w3qT = nc.dram_tensor("w3qT", [NP_D * P, 2 * D], F8, kind="ExternalInput")
    xgqT = nc.dram_tensor("xgqT", [NP_U * P, 2 * C], F8, kind="ExternalInput")

    shared_outT = nc.dram_tensor("shared_outT", [D, T], BF16, kind="ExternalOutput")
    routed_outT = nc.dram_tensor("routed_outT", [D, C], BF16, kind="ExternalOutput")

    # routed token blocks
    blk_w = [NT] * (C // NT)
    if C % NT:
        blk_w.append(C % NT)
    NBLK = len(blk_w)
    early_prefetch = C <= 1280

    with tile.TileContext(nc) as tc:
        with ExitStack() as ctx:
            # pools that live across both phases
            hpool = ctx.enter_context(tc.tile_pool(name="h_p", bufs=2))
            tpool = ctx.enter_context(tc.tile_pool(name="t_p", bufs=2))
            opool = ctx.enter_context(tc.tile_pool(name="o_p", bufs=4))
            # shared-phase first-half weights + chunk-0/sb-0 x: entered
            # before the routed pools (pool release is LIFO) so they
            # survive into the shared phase
            wsh1 = ctx.enter_context(tc.tile_pool(name="w_sh1", bufs=1))
            xA0 = ctx.enter_context(tc.tile_pool(name="x_a0", bufs=1))
            sw1_h1 = [wsh1.tile([P, H2], BF16, name=f"sw1_{k}_0",
                                tag=f"sw1_{k}_0") for k in range(KD)]
            sw2_h1 = [wsh1.tile([P, H2], BF16, name=f"sw2_{k}_0",
                                tag=f"sw2_{k}_0") for k in range(KD)]
            x00 = [xA0.tile([P, NT], BF16, name=f"xa_{k}", tag=f"xa_{k}")
                   for k in range(KD)]

            # input DMAs stay on the sync queue: the stream is DMA-ring
            # completion-limited (~wire speed), and the scalar queue must
            # stay clear for the silu chain
            def emit_prefetch_a():
                for k in range(KD):
                    nc.sync.dma_start(sw1_h1[k][:], sw1T[k * P:(k + 1) * P, :H2])
                    nc.sync.dma_start(sw2_h1[k][:], sw2T[k * P:(k + 1) * P, :H2])
                for k in range(KD):
                    nc.sync.dma_start(x00[k][:], xT[k * P:(k + 1) * P, 0:NT])

            # HAM warmup: dummy matmuls on memset SBUF keep the PE busy
            # through its cold-clock window while the first weights stream
            wmp = ctx.enter_context(tc.tile_pool(name="wm_p", bufs=1))
            wwm = wmp.tile([P, P], BF16, name="wwm", tag="wwm")
            xwm = wmp.tile([P, NT // 2], BF16, name="xwm", tag="xwm")
            nc.vector.memset(wwm[:], 0)
            nc.gpsimd.memset(xwm[:], 0)

            with ExitStack() as ctx_r:
                # ---------------- Phase R: routed expert (all fp8) -------
                wr = ctx_r.enter_context(tc.tile_pool(name="w_r", bufs=1))
                xgp = ctx_r.enter_context(tc.tile_pool(name="xg_p", bufs=1))
                # deep routed output ring: the fp8 down-proj produces so
                # tiles faster than the SWDGE drains them
                orp = ctx_r.enter_context(tc.tile_pool(name="or_p", bufs=8))
                hqp = ctx_r.enter_context(tc.tile_pool(name="hq_p", bufs=2))
                psR = ctx_r.enter_context(
                    tc.tile_pool(name="psR", bufs=1, space="PSUM"))

                for g in range(2):
                    pw = psR.tile([P, NT], F32, name="pb7", tag="pb7")
                    for k in range(5):
                        nc.tensor.matmul(pw[:, :NT // 2], wwm[:], xwm[:],
                                         start=(k == 0), stop=(k == 4))

                w12q_sb = [wr.tile([P, 2, 2 * HR], F8, name=f"w12q_{j}",
                                   tag=f"w12q_{j}") for j in range(NP_U)]
                w3q_sb = [wr.tile([P, 2, D], F8, name=f"w3q_{j}",
                                  tag=f"w3q_{j}") for j in range(NP_D)]
                xgq_sb = [xgp.tile([P, 2, C], F8, name=f"xgq_{j}",
                                   tag=f"xgq_{j}") for j in range(NP_U)]

                def w12q_src(j):
                    return w12qT[j * P:(j + 1) * P, :].rearrange(
                        "p (i c) -> p i c", i=2)

                def xgq_src(j):
                    return xgqT[j * P:(j + 1) * P, :].rearrange(
                        "p (i c) -> p i c", i=2)

                # DMA emission = execution order on the sync queue,
                # need-ordered: block-0 tokens + mg0 weight halves first
                nt0 = min(NT, C)
                for j in range(NP_U):
                    nc.sync.dma_start(xgq_sb[j][:, :, 0:nt0],
                                      xgq_src(j)[:, :, 0:nt0])
                    nc.sync.dma_start(w12q_sb[j][:, :, 0:HR],
                                      w12q_src(j)[:, :, 0:HR])
                for j in range(NP_U):
                    nc.sync.dma_start(w12q_sb[j][:, :, HR:],
                                      w12q_src(j)[:, :, HR:])
                for j in range(NP_D):
                    nc.sync.dma_start(w3q_sb[j][:, :, :],
                                      w3qT[j * P:(j + 1) * P, :].rearrange(
                                          "p (i c) -> p i c", i=2))
                if C > nt0:
                    for j in range(NP_U):
                        nc.sync.dma_start(xgq_sb[j][:, :, nt0:],
                                          xgq_src(j)[:, :, nt0:])

                if early_prefetch:
                    emit_prefetch_a()

                # routed compute
                off = 0
                for b, nt in enumerate(blk_w):
                    tok = slice(off, off + nt)
                    off += nt
                    # fine-grained first chunks: the first matmuls gate on
                    # one xgq+w12q pair so real work starts sooner
                    jchunks = ([range(0, 1), range(1, 2), range(2, 4),
                                range(4, 8)]
                               if b == 0 else [range(0, 4), range(4, 8)])
                    hq = [hqp.tile([P, 2, NT], F8, name=f"hq_{j}",
                                   tag=f"hq_{j}") for j in range(NP_D)]
                    for mg in range(2):
                        pg = [psR.tile([P, NT], F32, name=f"pb{m}", tag=f"pb{m}")
                              for m in range(4)]
                        pu = [psR.tile([P, NT], F32, name=f"pb{4+m}", tag=f"pb{4+m}")
                              for m in range(4)]
                        for js in jchunks:
                            for m in range(4):
                                c1 = slice(mg * HR + m * P, mg * HR + (m + 1) * P)
                                c2 = slice(mg * HR + NT + m * P,
                                           mg * HR + NT + (m + 1) * P)
                                for j in js:
                                    nc.tensor.matmul(pg[m][:, :nt],
                                                     w12q_sb[j][:, :, c1],
                                                     xgq_sb[j][:, :, tok],
                                                     start=(j == 0),
                                                     stop=(j == NP_U - 1),
                                                     perf_mode=DR)
                                for j in js:
                                    nc.tensor.matmul(pu[m][:, :nt],
                                                     w12q_sb[j][:, :, c2],
                                                     xgq_sb[j][:, :, tok],
                                                     start=(j == 0),
                                                     stop=(j == NP_U - 1),
                                                     perf_mode=DR)
                        for m in range(4):
                            sg = tpool.tile([P, NT], F32, name="sg", tag="sg")
                            nc.scalar.activation(sg[:, :nt], pg[m][:, :nt], AF.Silu)
                            # h (value h*0.5 via the w12 up-scale) written
                            # fp8 into its DoubleRow pair plane
                            hidx = mg * 4 + m
                            nc.vector.tensor_mul(hq[hidx // 2][:, hidx % 2, :nt],
                                                 sg[:, :nt], pu[m][:, :nt])
                    last_blk = (b == NBLK - 1)
                    for mo in range(KD):
                        # On the last block, flip the mo->PSUM-tag map: the
                        # shared phase's first psA tiles reallocate the
                        # last-freed banks, so hand those banks to the
                        # earliest-copied mo tiles
                        pb = (7 - mo % 8) if last_blk else (mo % 8)
                        po = psR.tile([P, NT], F32, name=f"pb{pb}", tag=f"pb{pb}")
                        for j in range(NP_D):
                            nc.tensor.matmul(po[:, :nt],
                                             w3q_sb[j][:, :, mo * P:(mo + 1) * P],
                                             hq[j][:, :, :nt],
                                             start=(j == 0), stop=(j == NP_D - 1),
                                             perf_mode=DR)
                        so = orp.tile([P, NT], BF16, name="so", tag="so")
                        nc.vector.tensor_copy(so[:, :nt], po[:, :nt])
                        # outputs split across the SWDGE and the scalar
                        # HWDGE: the fp8 down-proj outpaces what the SWDGE
                        # alone can drain, and sync must keep streaming
                        (nc.gpsimd if mo % 2 == 0 else nc.scalar).dma_start(
                            routed_outT[mo * P:(mo + 1) * P, tok], so[:, :nt])

            # ---------------- Phase S: shared expert (bf16) ----------
            wsh2 = ctx.enter_context(tc.tile_pool(name="w_sh2", bufs=1))
            wdn = ctx.enter_context(tc.tile_pool(name="w_dn", bufs=1))
            xpool = ctx.enter_context(tc.tile_pool(name="x_p", bufs=2))
            psA = ctx.enter_context(tc.tile_pool(name="psA", bufs=2, space="PSUM"))
            psB = ctx.enter_context(tc.tile_pool(name="psB", bufs=4, space="PSUM"))

            if not early_prefetch:
                emit_prefetch_a()

            # second weight halves, down-proj weights, chunk-0/sb-1 x
            sw1_h2 = [wsh2.tile([P, H2], BF16, name=f"sw1_{k}_1", tag=f"sw1_{k}_1")
                      for k in range(KD)]
            sw2_h2 = [wsh2.tile([P, H2], BF16, name=f"sw2_{k}_1", tag=f"sw2_{k}_1")
                      for k in range(KD)]
            sw3_sb = [wdn.tile([P, D], BF16, name=f"sw3_{k}", tag=f"sw3_{k}")
                      for k in range(KH)]
            x01 = [xpool.tile([P, NT], BF16, name=f"x_{k}_1", tag=f"x_{k}_1")
                   for k in range(KD)]
            for k in range(KD):
                nc.sync.dma_start(sw1_h2[k][:], sw1T[k * P:(k + 1) * P, H2:])
                nc.sync.dma_start(sw2_h2[k][:], sw2T[k * P:(k + 1) * P, H2:])
            for k in range(KH // 2):
                nc.sync.dma_start(sw3_sb[k][:], sw3T[k * P:(k + 1) * P, :])
            for k in range(KD // 2):
                nc.sync.dma_start(x01[k][:], xT[k * P:(k + 1) * P, NT:CH])
            for k in range(KH // 2, KH):
                nc.sync.dma_start(sw3_sb[k][:], sw3T[k * P:(k + 1) * P, :])
            for k in range(KD // 2, KD):
                nc.sync.dma_start(x01[k][:], xT[k * P:(k + 1) * P, NT:CH])

            sw_h = [[sw1_h1, sw1_h2], [sw2_h1, sw2_h2]]

            for ch in range(T // CH):
                if ch == 0:
                    x_sb = [[x00[k], x01[k]] for k in range(KD)]
                else:
                    x_sb = [[xpool.tile([P, NT], BF16, name=f"x_{k}_{h}",
                                        tag=f"x_{k}_{h}")
                             for h in range(CH // NT)] for k in range(KD)]
                    for k in range(KD):
                        for h in range(CH // NT):
                            nc.sync.dma_start(
                                x_sb[k][h][:],
                                xT[k * P:(k + 1) * P,
                                   ch * CH + h * NT:ch * CH + (h + 1) * NT])
                for sb in range(CH // NT):
                    otok = slice(ch * CH + sb * NT, ch * CH + (sb + 1) * NT)
                    hs = []
                    for m in range(KH):
                        wh, wm = divmod(m, H2 // P)   # which weight half-tile
                        mm = slice(wm * P, (wm + 1) * P)
                        pg = psA.tile([P, NT], F32, name="pg", tag="pg")
                        pu = psA.tile([P, NT], F32, name="pu", tag="pu")
                        for k in range(KD):
                            nc.tensor.matmul(pg[:], sw_h[0][wh][k][:, mm],
                                             x_sb[k][sb][:],
                                             start=(k == 0), stop=(k == KD - 1))
                        for k in range(KD):
                            nc.tensor.matmul(pu[:], sw_h[1][wh][k][:, mm],
                                             x_sb[k][sb][:],
                                             start=(k == 0), stop=(k == KD - 1))
                        sg = tpool.tile([P, NT], F32, name="sg", tag="sg")
                        nc.scalar.activation(sg[:], pg[:], AF.Silu)
                        h = hpool.tile([P, NT], BF16, name=f"h_{m}", tag=f"h_{m}")
                        nc.vector.tensor_mul(h[:], sg[:], pu[:])
                        hs.append(h)
                    last_sb = (ch == T // CH - 1 and sb == CH // NT - 1)
                    for mo in range(KD):
                        orow = slice(mo * P, (mo + 1) * P)
                        if last_sb and mo >= KD - 2:
                            # final two tiles in half-width groups: short
                            # post-matmul drain, DMAs over both HWDGE queues
                            dma_eng = [nc.sync, nc.scalar, nc.sync, nc.scalar]
                            for hf in range(2):
                                pi = (mo - (KD - 2)) * 2 + hf
                                cs = slice(hf * (NT // 2), (hf + 1) * (NT // 2))
                                po = psB.tile([P, NT], F32, name="po", tag="po")
                                for k in range(KH):
                                    nc.tensor.matmul(po[:, :NT // 2],
                                                     sw3_sb[k][:, orow],
                                                     hs[k][:, cs],
                                                     start=(k == 0), stop=(k == KH - 1))
                                so = opool.tile([P, NT], BF16, name="so", tag="so")
                                nc.vector.tensor_copy(so[:, :NT // 2],
                                                      po[:, :NT // 2])
                                dma_eng[pi].dma_start(
                                    shared_outT[orow,
                                                otok.start + hf * (NT // 2):
                                                otok.start + (hf + 1) * (NT // 2)],
                                    so[:, :NT // 2])
                            continue
                        po = psB.tile([P, NT], F32, name="po", tag="po")
                        for k in range(KH):
                            nc.tensor.matmul(po[:], sw3_sb[k][:, orow],
                                             hs[k][:],
                                             start=(k == 0), stop=(k == KH - 1))
                        so = opool.tile([P, NT], BF16, name="so", tag="so")
                        nc.vector.tensor_copy(so[:], po[:])
                        if ch == T // CH - 1:
                            # last chunk: outputs ride the two fast HWDGE
                            # queues so the slow SWDGE drains well before
                            # the end-of-kernel barrier
                            (nc.sync if mo % 2 == 0 else nc.scalar).dma_start(
                                shared_outT[orow, otok], so[:])
                        else:
                            # split with the scalar HWDGE to keep the SWDGE
                            # comfortably under its drain rate
                            (nc.gpsimd if mo % 2 == 0 else nc.scalar).dma_start(
                                shared_outT[orow, otok], so[:])

    nc.compile()
    return nc


_PROGRAM_CACHE: dict = {}


def _get_program(C: int):
    if C not in _PROGRAM_CACHE:
        _PROGRAM_CACHE[C] = _build_program(C)
    return _PROGRAM_CACHE[C]


def _route_like_reference(xf: np.ndarray, router_w: np.ndarray,
                          expert_bias: np.ndarray):
    """Router computed with jax on CPU to bit-match the reference's top-k."""
    import jax
    import jax.numpy as jnp

    cpu = jax.devices("cpu")[0]
    with jax.default_device(cpu):
        xj = jnp.asarray(xf)
        scores = jax.nn.sigmoid(xj @ jnp.asarray(router_w).T)        # (T, E)
        sel = scores + jnp.asarray(expert_bias)
        _, top_idx = jax.lax.top_k(sel, TOPK)                        # (T, K)
        top_sc = jnp.take_along_axis(scores, top_idx, axis=-1)
        top_w = top_sc / (top_sc.sum(-1, keepdims=True) + 1e-9)
        return np.asarray(top_idx), np.asarray(top_w)


def _pack_pairs(vals: np.ndarray, npairs: int) -> np.ndarray:
    """[npairs*2*P, C] fp32 -> e4m3 [npairs*P, 2*C] DoubleRow layout:
    row j*P+p holds [chunk(2j) row p | chunk(2j+1) row p]."""
    Cc = vals.shape[1]
    out = np.empty((npairs * P, 2 * Cc), dtype=E4M3)
    q = np.clip(vals, -240.0, 240.0).astype(E4M3)
    for j in range(npairs):
        out[j * P:(j + 1) * P, :Cc] = q[2 * j * P:(2 * j + 1) * P]
        out[j * P:(j + 1) * P, Cc:] = q[(2 * j + 1) * P:(2 * j + 2) * P]
    return out


def kernel(x, w12, w3, router_w, expert_bias, sw1, sw2, sw3):
    x = np.asarray(x, dtype=np.float32)
    w12 = np.asarray(w12, dtype=np.float32)
    w3 = np.asarray(w3, dtype=np.float32)
    router_w = np.asarray(router_w, dtype=np.float32)
    expert_bias = np.asarray(expert_bias, dtype=np.float32)
    sw1 = np.asarray(sw1, dtype=np.float32)
    sw2 = np.asarray(sw2, dtype=np.float32)
    sw3 = np.asarray(sw3, dtype=np.float32)

    xf = x.reshape(T, D)
    top_idx, top_w = _route_like_reference(xf, router_w, expert_bias)

    # per-expert token lists + combine weights
    idx_list, w_list = [], []
    for e in range(E):
        hit = top_idx == e                      # (T, K)
        tok = np.nonzero(hit.any(axis=1))[0]
        wt = (top_w * hit).sum(axis=1)[tok]     # combine weight per token
        idx_list.append(tok.astype(np.int64))
        w_list.append(wt.astype(np.float32))

    max_n = max(len(i) for i in idx_list)
    # Device capacity policy: cap at C_CORE (the exact mean load for top-2 of
    # 8 experts) and fix up small per-expert overflows on host in fp32.
    C_CORE = 1024
    C_MAX = 1280   # slab size for the imbalanced-routing fallback
    overflow = sum(max(0, len(i) - C_CORE) for i in idx_list)
    if max_n <= C_CORE:
        C = max(P, -(-max_n // P) * P)          # capacity, multiple of 128
        n_launches, host_fix = 1, False
    elif overflow <= 1024:
        C, n_launches, host_fix = C_CORE, 1, True
    else:
        C = C_MAX
        n_launches, host_fix = max(1, -(-max_n // C_MAX)), False

    xTf = np.ascontiguousarray(xf.T)                        # (D, T) fp32
    xT16 = xTf.astype(ml_dtypes.bfloat16)
    # fp8 x at value x*0.25 for the routed expert (all of D)
    xq8 = np.clip(xTf * 0.25, -240, 240).astype(E4M3)       # (D, T)

    nc = _get_program(C)

    outT = np.zeros((D, T), dtype=np.float32)
    global _LAST_RESULTS
    for launch in range(n_launches):
        lo = launch * C_MAX
        in_maps = []
        for c in range(NCORES):
            hs = slice(c * HC, (c + 1) * HC)
            idx_c = idx_list[c][lo:lo + C]
            n_c = len(idx_c)
            xgq_full = np.zeros((D, C), dtype=np.float32)
            if n_c:
                xgq_full[:, :n_c] = xq8[:, idx_c].astype(np.float32)
            xgq = _pack_pairs(xgq_full, NP_U)
            if launch == 0:
                s1 = np.ascontiguousarray(sw1[hs].T).astype(ml_dtypes.bfloat16)
                s2 = np.ascontiguousarray(sw2[hs].T).astype(ml_dtypes.bfloat16)
                s3 = np.ascontiguousarray(sw3[:, hs].T).astype(ml_dtypes.bfloat16)
            else:
                s1 = np.zeros((D, HC), dtype=ml_dtypes.bfloat16)
                s2 = s1
                s3 = np.zeros((HC, D), dtype=ml_dtypes.bfloat16)
            # w12 columns reordered [gate m0-3 | up m0-3 | gate m4-7 |
            # up m4-7]; gate x4, up x2 (x is at 0.25: gate true-scale,
            # u at 0.5 -> h lands at 0.5 for the fp8 down-proj)
            w12t = np.ascontiguousarray(w12[c].T)           # (D, 2HR) fp32
            w12rq = np.concatenate(
                [w12t[:, 0:NT] * 4.0, w12t[:, HR:HR + NT] * 2.0,
                 w12t[:, NT:HR] * 4.0, w12t[:, HR + NT:] * 2.0], axis=1)
            in_maps.append({
                "xT": xT16,
                "sw1T": s1, "sw2T": s2, "sw3T": s3,
                "w12qT": _pack_pairs(w12rq, NP_U),
                "w3qT": _pack_pairs(np.ascontiguousarray(w3[c].T) * 2.0, NP_D),
                "xgqT": xgq,
            })

        res = run_bass_kernel_spmd(nc, in_maps, core_ids=list(range(NCORES)),
                                   **_RUN_KWARGS)
        _LAST_RESULTS = res

        for c in range(NCORES):
            if launch == 0:
                outT += res.results[c]["shared_outT"].astype(np.float32)
            idx_c = idx_list[c][lo:lo + C]
            if len(idx_c):
                ro = res.results[c]["routed_outT"][:, :len(idx_c)].astype(np.float32)
                outT[:, idx_c] += ro * w_list[c][lo:lo + C][None, :]

    if host_fix:
        # fp32 fixup for tokens beyond the device capacity of each expert
        for c in range(NCORES):
            tail = idx_list[c][C:]
            if len(tail) == 0:
                continue
            wts = w_list[c][C:]
            xs = xf[tail]                             # (n, D)
            h12 = xs @ w12[c].T                       # (n, 2*HR)
            h1, h2 = h12[:, :HR], h12[:, HR:]
            h = h1 / (1.0 + np.exp(-h1)) * h2         # silu(h1) * h2
            out = (h * wts[:, None]) @ w3[c].T        # (n, D)
            outT[:, tail] += out.T
    return outT.T.reshape(B, S, D).astype(np.float32)


# test harness hooks: set _RUN_KWARGS = {"trace": True, ...} before calling
# kernel() to profile; read _LAST_RESULTS afterwards.
_RUN_KWARGS: dict = {}
_LAST_RESULTS = None


# revision 29
# speedup vs baseline: 1.1107x; 1.0020x over previous
"""MoE (shared expert + 8 routed experts, top-2) on 8 Trainium2 NeuronCores.

Sharding: core c holds
  - shared-expert slice c: rows [c*1024, (c+1)*1024) of sw1/sw2 and the
    matching columns of sw3  -> partial (T, D) output, summed on host
  - routed expert c's weights (w12[c], w3[c]); host routes/gathers the
    tokens selected for expert c (capacity 1024 = the exact mean load),
    device computes unscaled expert outputs, host applies combine weights
    during the fp32 scatter-add; small per-expert overflows beyond the
    capacity are fixed up on host in fp32.

Precision: the shared expert runs bf16 with fp32 PSUM (its 8-way
H-sharded contraction sums across cores, so even one fp8 DoubleRow pair
per slice puts 1/4 of the global contraction in fp8 — too noisy).  The
ROUTED expert runs entirely in fp8-e4m3 DoubleRow (2x PE throughput):
its per-expert contraction lives on one core, and full-fp8 routed costs
~1.0e-2 of the 2e-2 error budget.  Scale products are exactly 1
(x*0.25 @ w12*4 for gates; x*0.25 @ w12*2 -> u*0.5 -> h*0.5 fp8 from
the DVE mul; h*0.5 @ w3*2), so fp8 partial sums are true-scale and no
dequant exists anywhere.  Simulated end-to-end max-rel 1.62e-2.

Schedule: routed expert first (fp8 weights stream at t=0, phase is now
~2x shorter), shared-expert weights prefetch behind routed compute;
drain copies/DMAs spread across engines at phase boundaries and the
tail; routed outputs split SWDGE/scalar-HWDGE (the fp8 down-proj
outpaces the SWDGE alone).
"""

import sys

if "/opt/trn_rl_repo" not in sys.path:
    sys.path.insert(0, "/opt/trn_rl_repo")

from contextlib import ExitStack

import numpy as np
import ml_dtypes

import concourse.bass as bass
import concourse.tile as tile
from concourse import mybir, bacc
from concourse.bass_utils import run_bass_kernel_spmd

BF16 = mybir.dt.bfloat16
F32 = mybir.dt.float32
F8 = mybir.dt.float8e4
AF = mybir.ActivationFunctionType
DR = mybir.MatmulPerfMode.DoubleRow
E4M3 = ml_dtypes.float8_e4m3

# Problem shape (hardcoded per spec)
B, S, D = 2, 2048, 2048
T = B * S                  # 4096 tokens
E = 8                      # routed experts == n_cores
TOPK = 2
H_SHARED = 8192
HC = H_SHARED // 8         # shared-expert hidden slice per core
HR = 1024                  # routed expert hidden
NCORES = 8
NT = 512                   # token block (one PSUM bank at fp32)
P = 128
CH = 1024                  # shared-phase x chunk (2 sub-blocks)

KD = D // P                # 16 contraction tiles over D
KH = HC // P               # 8 contraction tiles over HC (== HR // P)
NP_U = 8                   # routed up-proj fp8 pairs (all of D)
NP_D = 4                   # routed down-proj fp8 pairs (all of HR)
H2 = HC // 2               # shared up-proj weight half-tile width


def _build_program(C: int):
    """SPMD Bass program, routed capacity C (multiple of 128)."""
    nc = bacc.Bacc("TRN2", target_bir_lowering=False, debug=False)

    xT = nc.dram_tensor("xT", [D, T], BF16, kind="ExternalInput")
    sw1T = nc.dram_tensor("sw1T", [D, HC], BF16, kind="ExternalInput")
    sw2T = nc.dram_tensor("sw2T", [D, HC], BF16, kind="ExternalInput")
    sw3T = nc.dram_tensor("sw3T", [HC, D], BF16, kind="ExternalInput")
    # fp8 routed tensors in DoubleRow pair layout: row j*128+p holds
    # [chunk(2j) | chunk(2j+1)] planes side by side.
    # w12q columns: [gate m0-3 | up m0-3 | gate m4-7 | up m4-7],
    # gate values w*4, up values w*2 (folds the h*0.5 scale)
    w12qT = nc.dram_tensor("w12qT", [NP_U * P, 2 * 2 * HR], F8, kind="ExternalInput")
    w3qT = nc.dram_tensor("w3qT", [NP_D * P, 2 * D], F8, kind="ExternalInput")
    xgqT = nc.dram_tensor("xgqT", [NP_U * P, 2 * C], F8, kind="ExternalInput")

    shared_outT = nc.dram_tensor("shared_outT", [D, T], BF16, kind="ExternalOutput")
    routed_outT = nc.dram_tensor("routed_outT", [D, C], BF16, kind="ExternalOutput")

    # routed token blocks
    blk_w = [NT] * (C // NT)
    if C % NT:
        blk_w.append(C % NT)
    NBLK = len(blk_w)
    early_prefetch = C <= 1280

    with tile.TileContext(nc) as tc:
        with ExitStack() as ctx:
            # pools that live across both phases
            hpool = ctx.enter_context(tc.tile_pool(name="h_p", bufs=2))
            tpool = ctx.enter_context(tc.tile_pool(name="t_p", bufs=2))
            opool = ctx.enter_context(tc.tile_pool(name="o_p", bufs=4))
            # shared-phase first-half weights + chunk-0/sb-0 x: entered
            # before the routed pools (pool release is LIFO) so they
            # survive into the shared phase
            wsh1 = ctx.enter_context(tc.tile_pool(name="w_sh1", bufs=1))
            xA0 = ctx.enter_context(tc.tile_pool(name="x_a0", bufs=1))
            sw1_h1 = [wsh1.tile([P, H2], BF16, name=f"sw1_{k}_0",
                                tag=f"sw1_{k}_0") for k in range(KD)]
            sw2_h1 = [wsh1.tile([P, H2], BF16, name=f"sw2_{k}_0",
                                tag=f"sw2_{k}_0") for k in range(KD)]
            x00 = [xA0.tile([P, NT], BF16, name=f"xa_{k}", tag=f"xa_{k}")
                   for k in range(KD)]

            # input DMAs stay on the sync queue: the stream is DMA-ring
            # completion-limited (~wire speed), and the scalar queue must
            # stay clear for the silu chain
            def emit_prefetch_a():
                for k in range(KD):
                    nc.sync.dma_start(sw1_h1[k][:], sw1T[k * P:(k + 1) * P, :H2])
                    nc.sync.dma_start(sw2_h1[k][:], sw2T[k * P:(k + 1) * P, :H2])
                for k in range(KD):
                    nc.sync.dma_start(x00[k][:], xT[k * P:(k + 1) * P, 0:NT])

            # HAM warmup: dummy matmuls on memset SBUF keep the PE busy
            # through its cold-clock window while the first weights stream
            wmp = ctx.enter_context(tc.tile_pool(name="wm_p", bufs=1))
            wwm = wmp.tile([P, P], BF16, name="wwm", tag="wwm")
            xwm = wmp.tile([P, NT // 2], BF16, name="xwm", tag="xwm")
            nc.vector.memset(wwm[:], 0)
            nc.gpsimd.memset(xwm[:], 0)

            with ExitStack() as ctx_r:
                # ---------------- Phase R: routed expert (all fp8) -------
                wr = ctx_r.enter_context(tc.tile_pool(name="w_r", bufs=1))
                xgp = ctx_r.enter_context(tc.tile_pool(name="xg_p", bufs=1))
                # deep routed output ring: the fp8 down-proj produces so
                # tiles faster than the SWDGE drains them
                orp = ctx_r.enter_context(tc.tile_pool(name="or_p", bufs=8))
                hqp = ctx_r.enter_context(tc.tile_pool(name="hq_p", bufs=2))
                psR = ctx_r.enter_context(
                    tc.tile_pool(name="psR", bufs=1, space="PSUM"))

                for g in range(2):
                    pw = psR.tile([P, NT], F32, name="pb7", tag="pb7")
                    for k in range(5):
                        nc.tensor.matmul(pw[:, :NT // 2], wwm[:], xwm[:],
                                         start=(k == 0), stop=(k == 4))

                w12q_sb = [wr.tile([P, 2, 2 * HR], F8, name=f"w12q_{j}",
                                   tag=f"w12q_{j}") for j in range(NP_U)]
                w3q_sb = [wr.tile([P, 2, D], F8, name=f"w3q_{j}",
                                  tag=f"w3q_{j}") for j in range(NP_D)]
                xgq_sb = [xgp.tile([P, 2, C], F8, name=f"xgq_{j}",
                                   tag=f"xgq_{j}") for j in range(NP_U)]

                def w12q_src(j):
                    return w12qT[j * P:(j + 1) * P, :].rearrange(
                        "p (i c) -> p i c", i=2)

                def xgq_src(j):
                    return xgqT[j * P:(j + 1) * P, :].rearrange(
                        "p (i c) -> p i c", i=2)

                # DMA emission = execution order on the sync queue,
                # need-ordered: block-0 tokens + mg0 weight halves first
                nt0 = min(NT, C)
                for j in range(NP_U):
                    nc.sync.dma_start(xgq_sb[j][:, :, 0:nt0],
                                      xgq_src(j)[:, :, 0:nt0])
                    nc.sync.dma_start(w12q_sb[j][:, :, 0:HR],
                                      w12q_src(j)[:, :, 0:HR])
                for j in range(NP_U):
                    nc.sync.dma_start(w12q_sb[j][:, :, HR:],
                                      w12q_src(j)[:, :, HR:])
                for j in range(NP_D):
                    nc.sync.dma_start(w3q_sb[j][:, :, :],
                                      w3qT[j * P:(j + 1) * P, :].rearrange(
                                          "p (i c) -> p i c", i=2))
                if C > nt0:
                    for j in range(NP_U):
                        nc.sync.dma_start(xgq_sb[j][:, :, nt0:],
                                          xgq_src(j)[:, :, nt0:])

                if early_prefetch:
                    emit_prefetch_a()

                # routed compute
                off = 0
                for b, nt in enumerate(blk_w):
                    tok = slice(off, off + nt)
                    off += nt
                    # fine-grained first chunks: the first matmuls gate on
                    # one xgq+w12q pair so real work starts sooner
                    jchunks = ([range(0, 1), range(1, 2), range(2, 4),
                                range(4, 8)]
                               if b == 0 else [range(0, 4), range(4, 8)])
                    hq = [hqp.tile([P, 2, NT], F8, name=f"hq_{j}",
                                   tag=f"hq_{j}") for j in range(NP_D)]
                    for mg in range(2):
                        pg = [psR.tile([P, NT], F32, name=f"pb{m}", tag=f"pb{m}")
                              for m in range(4)]
                        pu = [psR.tile([P, NT], F32, name=f"pb{4+m}", tag=f"pb{4+m}")
                              for m in range(4)]
                        for js in jchunks:
                            for m in range(4):
                                c1 = slice(mg * HR + m * P, mg * HR + (m + 1) * P)
                                c2 = slice(mg * HR + NT + m * P,
                                           mg * HR + NT + (m + 1) * P)
                                for j in js:
                                    nc.tensor.matmul(pg[m][:, :nt],
                                                     w12q_sb[j][:, :, c1],
                                                     xgq_sb[j][:, :, tok],
                                                     start=(j == 0),
                                                     stop=(j == NP_U - 1),
                                                     perf_mode=DR)
                                for j in js:
                                    nc.tensor.matmul(pu[m][:, :nt],
                                                     w12q_sb[j][:, :, c2],
                                                     xgq_sb[j][:, :, tok],
                                                     start=(j == 0),
                                                     stop=(j == NP_U - 1),
                                                     perf_mode=DR)
                        for m in range(4):
                            sg = tpool.tile([P, NT], F32, name="sg", tag="sg")
                            nc.scalar.activation(sg[:, :nt], pg[m][:, :nt], AF.Silu)
                            # h (value h*0.5 via the w12 up-scale) written
                            # fp8 into its DoubleRow pair plane
                            hidx = mg * 4 + m
                            nc.vector.tensor_mul(hq[hidx // 2][:, hidx % 2, :nt],
                                                 sg[:, :nt], pu[m][:, :nt])
                    last_blk = (b == NBLK - 1)
                    for mo in range(KD):
                        # On the last block, flip the mo->PSUM-tag map: the
                        # shared phase's first psA tiles reallocate the
                        # last-freed banks, so hand those banks to the
                        # earliest-copied mo tiles
                        pb = (7 - mo % 8) if last_blk else (mo % 8)
                        po = psR.tile([P, NT], F32, name=f"pb{pb}", tag=f"pb{pb}")
                        for j in range(NP_D):
                            nc.tensor.matmul(po[:, :nt],
                                             w3q_sb[j][:, :, mo * P:(mo + 1) * P],
                                             hq[j][:, :, :nt],
                                             start=(j == 0), stop=(j == NP_D - 1),
                                             perf_mode=DR)
                        so = orp.tile([P, NT], BF16, name="so", tag="so")
                        nc.vector.tensor_copy(so[:, :nt], po[:, :nt])
                        # all routed outputs ride the scalar HWDGE: the fp8
                        # down-proj outpaces the SWDGE (which must enter the
                        # shared phase with an empty queue — a routed
                        # backlog there stalls the shared so-ring), and
                        # sync must keep streaming weights
                        nc.scalar.dma_start(
                            routed_outT[mo * P:(mo + 1) * P, tok], so[:, :nt])

            # ---------------- Phase S: shared expert (bf16) ----------
            wsh2 = ctx.enter_context(tc.tile_pool(name="w_sh2", bufs=1))
            wdn = ctx.enter_context(tc.tile_pool(name="w_dn", bufs=1))
            xpool = ctx.enter_context(tc.tile_pool(name="x_p", bufs=2))
            psA = ctx.enter_context(tc.tile_pool(name="psA", bufs=2, space="PSUM"))
            psB = ctx.enter_context(tc.tile_pool(name="psB", bufs=4, space="PSUM"))

            if not early_prefetch:
                emit_prefetch_a()

            # second weight halves, down-proj weights, chunk-0/sb-1 x
            sw1_h2 = [wsh2.tile([P, H2], BF16, name=f"sw1_{k}_1", tag=f"sw1_{k}_1")
                      for k in range(KD)]
            sw2_h2 = [wsh2.tile([P, H2], BF16, name=f"sw2_{k}_1", tag=f"sw2_{k}_1")
                      for k in range(KD)]
            sw3_sb = [wdn.tile([P, D], BF16, name=f"sw3_{k}", tag=f"sw3_{k}")
                      for k in range(KH)]
            x01 = [xpool.tile([P, NT], BF16, name=f"x_{k}_1", tag=f"x_{k}_1")
                   for k in range(KD)]
            for k in range(KD):
                nc.sync.dma_start(sw1_h2[k][:], sw1T[k * P:(k + 1) * P, H2:])
                nc.sync.dma_start(sw2_h2[k][:], sw2T[k * P:(k + 1) * P, H2:])
            for k in range(KH // 2):
                nc.sync.dma_start(sw3_sb[k][:], sw3T[k * P:(k + 1) * P, :])
            for k in range(KD // 2):
                nc.sync.dma_start(x01[k][:], xT[k * P:(k + 1) * P, NT:CH])
            for k in range(KH // 2, KH):
                nc.sync.dma_start(sw3_sb[k][:], sw3T[k * P:(k + 1) * P, :])
            for k in range(KD // 2, KD):
                nc.sync.dma_start(x01[k][:], xT[k * P:(k + 1) * P, NT:CH])

            sw_h = [[sw1_h1, sw1_h2], [sw2_h1, sw2_h2]]

            for ch in range(T // CH):
                if ch == 0:
                    x_sb = [[x00[k], x01[k]] for k in range(KD)]
                else:
                    x_sb = [[xpool.tile([P, NT], BF16, name=f"x_{k}_{h}",
                                        tag=f"x_{k}_{h}")
                             for h in range(CH // NT)] for k in range(KD)]
                    for k in range(KD):
                        for h in range(CH // NT):
                            nc.sync.dma_start(
                                x_sb[k][h][:],
                                xT[k * P:(k + 1) * P,
                                   ch * CH + h * NT:ch * CH + (h + 1) * NT])
                for sb in range(CH // NT):
                    otok = slice(ch * CH + sb * NT, ch * CH + (sb + 1) * NT)
                    hs = []
                    for m in range(KH):
                        wh, wm = divmod(m, H2 // P)   # which weight half-tile
                        mm = slice(wm * P, (wm + 1) * P)
                        pg = psA.tile([P, NT], F32, name="pg", tag="pg")
                        pu = psA.tile([P, NT], F32, name="pu", tag="pu")
                        for k in range(KD):
                            nc.tensor.matmul(pg[:], sw_h[0][wh][k][:, mm],
                                             x_sb[k][sb][:],
                                             start=(k == 0), stop=(k == KD - 1))
                        for k in range(KD):
                            nc.tensor.matmul(pu[:], sw_h[1][wh][k][:, mm],
                                             x_sb[k][sb][:],
                                             start=(k == 0), stop=(k == KD - 1))
                        sg = tpool.tile([P, NT], F32, name="sg", tag="sg")
                        nc.scalar.activation(sg[:], pg[:], AF.Silu)
                        h = hpool.tile([P, NT], BF16, name=f"h_{m}", tag=f"h_{m}")
                        nc.vector.tensor_mul(h[:], sg[:], pu[:])
                        hs.append(h)
                    last_sb = (ch == T // CH - 1 and sb == CH // NT - 1)
                    for mo in range(KD):
                        orow = slice(mo * P, (mo + 1) * P)
                        if last_sb and mo >= KD - 2:
                            # final two tiles in half-width groups: short
                            # post-matmul drain, DMAs over both HWDGE queues
                            dma_eng = [nc.sync, nc.scalar, nc.sync, nc.scalar]
                            for hf in range(2):
                                pi = (mo - (KD - 2)) * 2 + hf
                                cs = slice(hf * (NT // 2), (hf + 1) * (NT // 2))
                                po = psB.tile([P, NT], F32, name="po", tag="po")
                                for k in range(KH):
                                    nc.tensor.matmul(po[:, :NT // 2],
                                                     sw3_sb[k][:, orow],
                                                     hs[k][:, cs],
                                                     start=(k == 0), stop=(k == KH - 1))
                                so = opool.tile([P, NT], BF16, name="so", tag="so")
                                nc.vector.tensor_copy(so[:, :NT // 2],
                                                      po[:, :NT // 2])
                                dma_eng[pi].dma_start(
                                    shared_outT[orow,
                                                otok.start + hf * (NT // 2):
                                                otok.start + (hf + 1) * (NT // 2)],
                                    so[:, :NT // 2])
                            continue
                        po = psB.tile([P, NT], F32, name="po", tag="po")
                        for k in range(KH):
                            nc.tensor.matmul(po[:], sw3_sb[k][:, orow],
                                             hs[k][:],
                                             start=(k == 0), stop=(k == KH - 1))
                        so = opool.tile([P, NT], BF16, name="so", tag="so")
                        nc.vector.tensor_copy(so[:], po[:])
                        if ch == T // CH - 1:
                            # last chunk: outputs ride the two fast HWDGE
                            # queues so the slow SWDGE drains well before
                            # the end-of-kernel barrier
                            (nc.sync if mo % 2 == 0 else nc.scalar).dma_start(
                                shared_outT[orow, otok], so[:])
                        else:
                            # split with the scalar HWDGE to keep the SWDGE
                            # comfortably under its drain rate
                            (nc.gpsimd if mo % 2 == 0 else nc.scalar).dma_start(
                                shared_outT[orow, otok], so[:])

    nc.compile()
    return nc


_PROGRAM_CACHE: dict = {}


def _get_program(C: int):
    if C not in _PROGRAM_CACHE:
        _PROGRAM_CACHE[C] = _build_program(C)
    return _PROGRAM_CACHE[C]


def _route_like_reference(xf: np.ndarray, router_w: np.ndarray,
                          expert_bias: np.ndarray):
    """Router computed with jax on CPU to bit-match the reference's top-k."""
    import jax
    import jax.numpy as jnp

    cpu = jax.devices("cpu")[0]
    with jax.default_device(cpu):
        xj = jnp.asarray(xf)
        scores = jax.nn.sigmoid(xj @ jnp.asarray(router_w).T)        # (T, E)
        sel = scores + jnp.asarray(expert_bias)
        _, top_idx = jax.lax.top_k(sel, TOPK)                        # (T, K)
        top_sc = jnp.take_along_axis(scores, top_idx, axis=-1)
        top_w = top_sc / (top_sc.sum(-1, keepdims=True) + 1e-9)
        return np.asarray(top_idx), np.asarray(top_w)


def _pack_pairs(vals: np.ndarray, npairs: int) -> np.ndarray:
    """[npairs*2*P, C] fp32 -> e4m3 [npairs*P, 2*C] DoubleRow layout:
    row j*P+p holds [chunk(2j) row p | chunk(2j+1) row p]."""
    Cc = vals.shape[1]
    out = np.empty((npairs * P, 2 * Cc), dtype=E4M3)
    q = np.clip(vals, -240.0, 240.0).astype(E4M3)
    for j in range(npairs):
        out[j * P:(j + 1) * P, :Cc] = q[2 * j * P:(2 * j + 1) * P]
        out[j * P:(j + 1) * P, Cc:] = q[(2 * j + 1) * P:(2 * j + 2) * P]
    return out


def kernel(x, w12, w3, router_w, expert_bias, sw1, sw2, sw3):
    x = np.asarray(x, dtype=np.float32)
    w12 = np.asarray(w12, dtype=np.float32)
    w3 = np.asarray(w3, dtype=np.float32)
    router_w = np.asarray(router_w, dtype=np.float32)
    expert_bias = np.asarray(expert_bias, dtype=np.float32)
    sw1 = np.asarray(sw1, dtype=np.float32)
    sw2 = np.asarray(sw2, dtype=np.float32)
    sw3 = np.asarray(sw3, dtype=np.float32)

    xf = x.reshape(T, D)
    top_idx, top_w = _route_like_reference(xf, router_w, expert_bias)

    # per-expert token lists + combine weights
    idx_list, w_list = [], []
    for e in range(E):
        hit = top_idx == e                      # (T, K)
        tok = np.nonzero(hit.any(axis=1))[0]
        wt = (top_w * hit).sum(axis=1)[tok]     # combine weight per token
        idx_list.append(tok.astype(np.int64))
        w_list.append(wt.astype(np.float32))

    max_n = max(len(i) for i in idx_list)
    # Device capacity policy: cap at C_CORE (the exact mean load for top-2 of
    # 8 experts) and fix up small per-expert overflows on host in fp32.
    C_CORE = 1024
    C_MAX = 1280   # slab size for the imbalanced-routing fallback
    overflow = sum(max(0, len(i) - C_CORE) for i in idx_list)
    if max_n <= C_CORE:
        C = max(P, -(-max_n // P) * P)          # capacity, multiple of 128
        n_launches, host_fix = 1, False
    elif overflow <= 1024:
        C, n_launches, host_fix = C_CORE, 1, True
    else:
        C = C_MAX
        n_launches, host_fix = max(1, -(-max_n // C_MAX)), False

    xTf = np.ascontiguousarray(xf.T)                        # (D, T) fp32
    xT16 = xTf.astype(ml_dtypes.bfloat16)
    # fp8 x at value x*0.25 for the routed expert (all of D)
    xq8 = np.clip(xTf * 0.25, -240, 240).astype(E4M3)       # (D, T)

    nc = _get_program(C)

    outT = np.zeros((D, T), dtype=np.float32)
    global _LAST_RESULTS
    for launch in range(n_launches):
        lo = launch * C_MAX
        in_maps = []
        for c in range(NCORES):
            hs = slice(c * HC, (c + 1) * HC)
            idx_c = idx_list[c][lo:lo + C]
            n_c = len(idx_c)
            xgq_full = np.zeros((D, C), dtype=np.float32)
            if n_c:
                xgq_full[:, :n_c] = xq8[:, idx_c].astype(np.float32)
            xgq = _pack_pairs(xgq_full, NP_U)
            if launch == 0:
                s1 = np.ascontiguousarray(sw1[hs].T).astype(ml_dtypes.bfloat16)
                s2 = np.ascontiguousarray(sw2[hs].T).astype(ml_dtypes.bfloat16)
                s3 = np.ascontiguousarray(sw3[:, hs].T).astype(ml_dtypes.bfloat16)
            else:
                s1 = np.zeros((D, HC), dtype=ml_dtypes.bfloat16)
                s2 = s1
                s3 = np.zeros((HC, D), dtype=ml_dtypes.bfloat16)
            # w12 columns reordered [gate m0-3 | up m0-3 | gate m4-7 |
            # up m4-7]; gate x4, up x2 (x is at 0.25: gate true-scale,
            # u at 0.5 -> h lands at 0.5 for the fp8 down-proj)
            w12t = np.ascontiguousarray(w12[c].T)           # (D, 2HR) fp32
            w12rq = np.concatenate(
                [w12t[:, 0:NT] * 4.0, w12t[:, HR:HR + NT] * 2.0,
                 w12t[:, NT:HR] * 4.0, w12t[:, HR + NT:] * 2.0], axis=1)
            in_maps.append({
                "xT": xT16,
                "sw1T": s1, "sw2T": s2, "sw3T": s3,
                "w12qT": _pack_pairs(w12rq, NP_U),
                "w3qT": _pack_pairs(np.ascontiguousarray(w3[c].T) * 2.0, NP_D),
                "xgqT": xgq,
            })

        res = run_bass_kernel_spmd(nc, in_maps, core_ids=list(range(NCORES)),
                                   **_RUN_KWARGS)
        _LAST_RESULTS = res

        for c in range(NCORES):
            if launch == 0:
                outT += res.results[c]["shared_outT"].astype(np.float32)
            idx_c = idx_list[c][lo:lo + C]
            if len(idx_c):
                ro = res.results[c]["routed_outT"][:, :len(idx_c)].astype(np.float32)
                outT[:, idx_c] += ro * w_list[c][lo:lo + C][None, :]

    if host_fix:
        # fp32 fixup for tokens beyond the device capacity of each expert
        for c in range(NCORES):
            tail = idx_list[c][C:]
            if len(tail) == 0:
                continue
            wts = w_list[c][C:]
            xs = xf[tail]                             # (n, D)
            h12 = xs @ w12[c].T                       # (n, 2*HR)
            h1, h2 = h12[:, :HR], h12[:, HR:]
            h = h1 / (1.0 + np.exp(-h1)) * h2         # silu(h1) * h2
            out = (h * wts[:, None]) @ w3[c].T        # (n, D)
            outT[:, tail] += out.T
    return outT.T.reshape(B, S, D).astype(np.float32)


# test harness hooks: set _RUN_KWARGS = {"trace": True, ...} before calling
# kernel() to profile; read _LAST_RESULTS afterwards.
_RUN_KWARGS: dict = {}
_LAST_RESULTS = None


# revision 30
# speedup vs baseline: 1.1140x; 1.0030x over previous
"""MoE (shared expert + 8 routed experts, top-2) on 8 Trainium2 NeuronCores.

Sharding: core c holds
  - shared-expert slice c: rows [c*1024, (c+1)*1024) of sw1/sw2 and the
    matching columns of sw3  -> partial (T, D) output, summed on host
  - routed expert c's weights (w12[c], w3[c]); host routes/gathers the
    tokens selected for expert c (capacity 1024 = the exact mean load),
    device computes unscaled expert outputs, host applies combine weights
    during the fp32 scatter-add; small per-expert overflows beyond the
    capacity are fixed up on host in fp32.

Precision: the shared expert runs bf16 with fp32 PSUM (its 8-way
H-sharded contraction sums across cores, so even one fp8 DoubleRow pair
per slice puts 1/4 of the global contraction in fp8 — too noisy).  The
ROUTED expert runs entirely in fp8-e4m3 DoubleRow (2x PE throughput):
its per-expert contraction lives on one core, and full-fp8 routed costs
~1.0e-2 of the 2e-2 error budget.  Scale products are exactly 1
(x*0.25 @ w12*4 for gates; x*0.25 @ w12*2 -> u*0.5 -> h*0.5 fp8 from
the DVE mul; h*0.5 @ w3*2), so fp8 partial sums are true-scale and no
dequant exists anywhere.  Simulated end-to-end max-rel 1.62e-2.

Schedule: routed expert first (fp8 weights stream at t=0, phase is now
~2x shorter), shared-expert weights prefetch behind routed compute;
drain copies/DMAs spread across engines at phase boundaries and the
tail; routed outputs split SWDGE/scalar-HWDGE (the fp8 down-proj
outpaces the SWDGE alone).
"""

import sys

if "/opt/trn_rl_repo" not in sys.path:
    sys.path.insert(0, "/opt/trn_rl_repo")

from contextlib import ExitStack

import numpy as np
import ml_dtypes

import concourse.bass as bass
import concourse.tile as tile
from concourse import mybir, bacc
from concourse.bass_utils import run_bass_kernel_spmd

BF16 = mybir.dt.bfloat16
F32 = mybir.dt.float32
F8 = mybir.dt.float8e4
AF = mybir.ActivationFunctionType
DR = mybir.MatmulPerfMode.DoubleRow
E4M3 = ml_dtypes.float8_e4m3

# Problem shape (hardcoded per spec)
B, S, D = 2, 2048, 2048
T = B * S                  # 4096 tokens
E = 8                      # routed experts == n_cores
TOPK = 2
H_SHARED = 8192
HC = H_SHARED // 8         # shared-expert hidden slice per core
HR = 1024                  # routed expert hidden
NCORES = 8
NT = 512                   # token block (one PSUM bank at fp32)
P = 128
CH = 1024                  # shared-phase x chunk (2 sub-blocks)

KD = D // P                # 16 contraction tiles over D
KH = HC // P               # 8 contraction tiles over HC (== HR // P)
NP_U = 8                   # routed up-proj fp8 pairs (all of D)
NP_D = 4                   # routed down-proj fp8 pairs (all of HR)
H2 = HC // 2               # shared up-proj weight half-tile width


def _build_program(C: int):
    """SPMD Bass program, routed capacity C (multiple of 128)."""
    nc = bacc.Bacc("TRN2", target_bir_lowering=False, debug=False)

    xT = nc.dram_tensor("xT", [D, T], BF16, kind="ExternalInput")
    sw1T = nc.dram_tensor("sw1T", [D, HC], BF16, kind="ExternalInput")
    sw2T = nc.dram_tensor("sw2T", [D, HC], BF16, kind="ExternalInput")
    sw3T = nc.dram_tensor("sw3T", [HC, D], BF16, kind="ExternalInput")
    # fp8 routed tensors in DoubleRow pair layout: row j*128+p holds
    # [chunk(2j) | chunk(2j+1)] planes side by side.
    # w12q columns: [gate m0-3 | up m0-3 | gate m4-7 | up m4-7],
    # gate values w*4, up values w*2 (folds the h*0.5 scale)
    w12qT = nc.dram_tensor("w12qT", [NP_U * P, 2 * 2 * HR], F8, kind="ExternalInput")
    w3qT = nc.dram_tensor("w3qT", [NP_D * P, 2 * D], F8, kind="ExternalInput")
    xgqT = nc.dram_tensor("xgqT", [NP_U * P, 2 * C], F8, kind="ExternalInput")

    shared_outT = nc.dram_tensor("shared_outT", [D, T], BF16, kind="ExternalOutput")
    routed_outT = nc.dram_tensor("routed_outT", [D, C], BF16, kind="ExternalOutput")

    # routed token blocks
    blk_w = [NT] * (C // NT)
    if C % NT:
        blk_w.append(C % NT)
    NBLK = len(blk_w)
    early_prefetch = C <= 1280

    with tile.TileContext(nc) as tc:
        with ExitStack() as ctx:
            # pools that live across both phases
            hpool = ctx.enter_context(tc.tile_pool(name="h_p", bufs=2))
            tpool = ctx.enter_context(tc.tile_pool(name="t_p", bufs=2))
            opool = ctx.enter_context(tc.tile_pool(name="o_p", bufs=4))
            # shared-phase first-half weights + chunk-0/sb-0 x: entered
            # before the routed pools (pool release is LIFO) so they
            # survive into the shared phase
            wsh1 = ctx.enter_context(tc.tile_pool(name="w_sh1", bufs=1))
            xA0 = ctx.enter_context(tc.tile_pool(name="x_a0", bufs=1))
            sw1_h1 = [wsh1.tile([P, H2], BF16, name=f"sw1_{k}_0",
                                tag=f"sw1_{k}_0") for k in range(KD)]
            sw2_h1 = [wsh1.tile([P, H2], BF16, name=f"sw2_{k}_0",
                                tag=f"sw2_{k}_0") for k in range(KD)]
            x00 = [xA0.tile([P, NT], BF16, name=f"xa_{k}", tag=f"xa_{k}")
                   for k in range(KD)]

            # input DMAs stay on the sync queue: the stream is DMA-ring
            # completion-limited (~wire speed), and the scalar queue must
            # stay clear for the silu chain
            def emit_prefetch_a():
                for k in range(KD):
                    nc.sync.dma_start(sw1_h1[k][:], sw1T[k * P:(k + 1) * P, :H2])
                    nc.sync.dma_start(sw2_h1[k][:], sw2T[k * P:(k + 1) * P, :H2])
                for k in range(KD):
                    nc.sync.dma_start(x00[k][:], xT[k * P:(k + 1) * P, 0:NT])

            # HAM warmup: dummy matmuls on memset SBUF keep the PE busy
            # through its cold-clock window while the first weights stream
            wmp = ctx.enter_context(tc.tile_pool(name="wm_p", bufs=1))
            wwm = wmp.tile([P, P], BF16, name="wwm", tag="wwm")
            xwm = wmp.tile([P, NT // 2], BF16, name="xwm", tag="xwm")
            nc.vector.memset(wwm[:], 0)
            nc.gpsimd.memset(xwm[:], 0)

            with ExitStack() as ctx_r:
                # ---------------- Phase R: routed expert (all fp8) -------
                wr = ctx_r.enter_context(tc.tile_pool(name="w_r", bufs=1))
                xgp = ctx_r.enter_context(tc.tile_pool(name="xg_p", bufs=1))
                # deep routed output ring: the fp8 down-proj produces so
                # tiles faster than the SWDGE drains them
                orp = ctx_r.enter_context(tc.tile_pool(name="or_p", bufs=8))
                hqp = ctx_r.enter_context(tc.tile_pool(name="hq_p", bufs=2))
                psR = ctx_r.enter_context(
                    tc.tile_pool(name="psR", bufs=1, space="PSUM"))

                for g in range(2):
                    pw = psR.tile([P, NT], F32, name="pb7", tag="pb7")
                    for k in range(5):
                        nc.tensor.matmul(pw[:, :NT // 2], wwm[:], xwm[:],
                                         start=(k == 0), stop=(k == 4))

                w12q_sb = [wr.tile([P, 2, 2 * HR], F8, name=f"w12q_{j}",
                                   tag=f"w12q_{j}") for j in range(NP_U)]
                w3q_sb = [wr.tile([P, 2, D], F8, name=f"w3q_{j}",
                                  tag=f"w3q_{j}") for j in range(NP_D)]
                xgq_sb = [xgp.tile([P, 2, C], F8, name=f"xgq_{j}",
                                   tag=f"xgq_{j}") for j in range(NP_U)]

                def w12q_src(j):
                    return w12qT[j * P:(j + 1) * P, :].rearrange(
                        "p (i c) -> p i c", i=2)

                def xgq_src(j):
                    return xgqT[j * P:(j + 1) * P, :].rearrange(
                        "p (i c) -> p i c", i=2)

                # DMA emission = execution order on the sync queue,
                # need-ordered: block-0 tokens + mg0 weight halves first
                nt0 = min(NT, C)
                for j in range(NP_U):
                    nc.sync.dma_start(xgq_sb[j][:, :, 0:nt0],
                                      xgq_src(j)[:, :, 0:nt0])
                    nc.sync.dma_start(w12q_sb[j][:, :, 0:HR],
                                      w12q_src(j)[:, :, 0:HR])
                for j in range(NP_U):
                    nc.sync.dma_start(w12q_sb[j][:, :, HR:],
                                      w12q_src(j)[:, :, HR:])
                for j in range(NP_D):
                    nc.sync.dma_start(w3q_sb[j][:, :, :],
                                      w3qT[j * P:(j + 1) * P, :].rearrange(
                                          "p (i c) -> p i c", i=2))
                if C > nt0:
                    for j in range(NP_U):
                        nc.sync.dma_start(xgq_sb[j][:, :, nt0:],
                                          xgq_src(j)[:, :, nt0:])

                if early_prefetch:
                    emit_prefetch_a()

                # routed compute
                off = 0
                for b, nt in enumerate(blk_w):
                    tok = slice(off, off + nt)
                    off += nt
                    # fine-grained first chunks: the first matmuls gate on
                    # one xgq+w12q pair so real work starts sooner
                    jchunks = ([range(0, 1), range(1, 2), range(2, 4),
                                range(4, 8)]
                               if b == 0 else [range(0, 4), range(4, 8)])
                    hq = [hqp.tile([P, 2, NT], F8, name=f"hq_{j}",
                                   tag=f"hq_{j}") for j in range(NP_D)]
                    for mg in range(2):
                        pg = [psR.tile([P, NT], F32, name=f"pb{m}", tag=f"pb{m}")
                              for m in range(4)]
                        pu = [psR.tile([P, NT], F32, name=f"pb{4+m}", tag=f"pb{4+m}")
                              for m in range(4)]
                        for js in jchunks:
                            for m in range(4):
                                c1 = slice(mg * HR + m * P, mg * HR + (m + 1) * P)
                                c2 = slice(mg * HR + NT + m * P,
                                           mg * HR + NT + (m + 1) * P)
                                for j in js:
                                    nc.tensor.matmul(pg[m][:, :nt],
                                                     w12q_sb[j][:, :, c1],
                                                     xgq_sb[j][:, :, tok],
                                                     start=(j == 0),
                                                     stop=(j == NP_U - 1),
                                                     perf_mode=DR)
                                for j in js:
                                    nc.tensor.matmul(pu[m][:, :nt],
                                                     w12q_sb[j][:, :, c2],
                                                     xgq_sb[j][:, :, tok],
                                                     start=(j == 0),
                                                     stop=(j == NP_U - 1),
                                                     perf_mode=DR)
                        for m in range(4):
                            sg = tpool.tile([P, NT], F32, name="sg", tag="sg")
                            nc.scalar.activation(sg[:, :nt], pg[m][:, :nt], AF.Silu)
                            # h (value h*0.5 via the w12 up-scale) written
                            # fp8 into its DoubleRow pair plane
                            hidx = mg * 4 + m
                            nc.vector.tensor_mul(hq[hidx // 2][:, hidx % 2, :nt],
                                                 sg[:, :nt], pu[m][:, :nt])
                    last_blk = (b == NBLK - 1)
                    for mo in range(KD):
                        # On the last block, flip the mo->PSUM-tag map: the
                        # shared phase's first psA tiles reallocate the
                        # last-freed banks, so hand those banks to the
                        # earliest-copied mo tiles
                        pb = (7 - mo % 8) if last_blk else (mo % 8)
                        po = psR.tile([P, NT], F32, name=f"pb{pb}", tag=f"pb{pb}")
                        for j in range(NP_D):
                            nc.tensor.matmul(po[:, :nt],
                                             w3q_sb[j][:, :, mo * P:(mo + 1) * P],
                                             hq[j][:, :, :nt],
                                             start=(j == 0), stop=(j == NP_D - 1),
                                             perf_mode=DR)
                        so = orp.tile([P, NT], BF16, name="so", tag="so")
                        nc.vector.tensor_copy(so[:, :nt], po[:, :nt])
                        # routed outputs split SWDGE/scalar-HWDGE: the fp8
                        # down-proj bursts ~51GB/s of strided DRAM writes,
                        # about 2x what one output queue drains
                        (nc.gpsimd if mo % 2 == 0 else nc.scalar).dma_start(
                            routed_outT[mo * P:(mo + 1) * P, tok], so[:, :nt])

            # ---------------- Phase S: shared expert (bf16) ----------
            wsh2 = ctx.enter_context(tc.tile_pool(name="w_sh2", bufs=1))
            wdn = ctx.enter_context(tc.tile_pool(name="w_dn", bufs=1))
            xpool = ctx.enter_context(tc.tile_pool(name="x_p", bufs=2))
            psA = ctx.enter_context(tc.tile_pool(name="psA", bufs=2, space="PSUM"))
            psB = ctx.enter_context(tc.tile_pool(name="psB", bufs=4, space="PSUM"))

            if not early_prefetch:
                emit_prefetch_a()

            # second weight halves, down-proj weights, chunk-0/sb-1 x
            sw1_h2 = [wsh2.tile([P, H2], BF16, name=f"sw1_{k}_1", tag=f"sw1_{k}_1")
                      for k in range(KD)]
            sw2_h2 = [wsh2.tile([P, H2], BF16, name=f"sw2_{k}_1", tag=f"sw2_{k}_1")
                      for k in range(KD)]
            sw3_sb = [wdn.tile([P, D], BF16, name=f"sw3_{k}", tag=f"sw3_{k}")
                      for k in range(KH)]
            x01 = [xpool.tile([P, NT], BF16, name=f"x_{k}_1", tag=f"x_{k}_1")
                   for k in range(KD)]
            for k in range(KD):
                nc.sync.dma_start(sw1_h2[k][:], sw1T[k * P:(k + 1) * P, H2:])
                nc.sync.dma_start(sw2_h2[k][:], sw2T[k * P:(k + 1) * P, H2:])
            for k in range(KH // 2):
                nc.sync.dma_start(sw3_sb[k][:], sw3T[k * P:(k + 1) * P, :])
            for k in range(KD // 2):
                nc.sync.dma_start(x01[k][:], xT[k * P:(k + 1) * P, NT:CH])
            for k in range(KH // 2, KH):
                nc.sync.dma_start(sw3_sb[k][:], sw3T[k * P:(k + 1) * P, :])
            for k in range(KD // 2, KD):
                nc.sync.dma_start(x01[k][:], xT[k * P:(k + 1) * P, NT:CH])

            sw_h = [[sw1_h1, sw1_h2], [sw2_h1, sw2_h2]]

            for ch in range(T // CH):
                if ch == 0:
                    x_sb = [[x00[k], x01[k]] for k in range(KD)]
                else:
                    x_sb = [[xpool.tile([P, NT], BF16, name=f"x_{k}_{h}",
                                        tag=f"x_{k}_{h}")
                             for h in range(CH // NT)] for k in range(KD)]
                    for k in range(KD):
                        for h in range(CH // NT):
                            nc.sync.dma_start(
                                x_sb[k][h][:],
                                xT[k * P:(k + 1) * P,
                                   ch * CH + h * NT:ch * CH + (h + 1) * NT])
                for sb in range(CH // NT):
                    otok = slice(ch * CH + sb * NT, ch * CH + (sb + 1) * NT)
                    hs = []
                    for m in range(KH):
                        wh, wm = divmod(m, H2 // P)   # which weight half-tile
                        mm = slice(wm * P, (wm + 1) * P)
                        pg = psA.tile([P, NT], F32, name="pg", tag="pg")
                        pu = psA.tile([P, NT], F32, name="pu", tag="pu")
                        for k in range(KD):
                            nc.tensor.matmul(pg[:], sw_h[0][wh][k][:, mm],
                                             x_sb[k][sb][:],
                                             start=(k == 0), stop=(k == KD - 1))
                        for k in range(KD):
                            nc.tensor.matmul(pu[:], sw_h[1][wh][k][:, mm],
                                             x_sb[k][sb][:],
                                             start=(k == 0), stop=(k == KD - 1))
                        sg = tpool.tile([P, NT], F32, name="sg", tag="sg")
                        nc.scalar.activation(sg[:], pg[:], AF.Silu)
                        h = hpool.tile([P, NT], BF16, name=f"h_{m}", tag=f"h_{m}")
                        nc.vector.tensor_mul(h[:], sg[:], pu[:])
                        hs.append(h)
                    last_sb = (ch == T // CH - 1 and sb == CH // NT - 1)
                    for mo in range(KD):
                        orow = slice(mo * P, (mo + 1) * P)
                        if last_sb and mo >= KD - 2:
                            # final two tiles in half-width groups: short
                            # post-matmul drain, DMAs over both HWDGE queues
                            dma_eng = [nc.sync, nc.scalar, nc.sync, nc.scalar]
                            for hf in range(2):
                                pi = (mo - (KD - 2)) * 2 + hf
                                cs = slice(hf * (NT // 2), (hf + 1) * (NT // 2))
                                po = psB.tile([P, NT], F32, name="po", tag="po")
                                for k in range(KH):
                                    nc.tensor.matmul(po[:, :NT // 2],
                                                     sw3_sb[k][:, orow],
                                                     hs[k][:, cs],
                                                     start=(k == 0), stop=(k == KH - 1))
                                so = opool.tile([P, NT], BF16, name="so", tag="so")
                                nc.vector.tensor_copy(so[:, :NT // 2],
                                                      po[:, :NT // 2])
                                dma_eng[pi].dma_start(
                                    shared_outT[orow,
                                                otok.start + hf * (NT // 2):
                                                otok.start + (hf + 1) * (NT // 2)],
                                    so[:, :NT // 2])
                            continue
                        po = psB.tile([P, NT], F32, name="po", tag="po")
                        for k in range(KH):
                            nc.tensor.matmul(po[:], sw3_sb[k][:, orow],
                                             hs[k][:],
                                             start=(k == 0), stop=(k == KH - 1))
                        so = opool.tile([P, NT], BF16, name="so", tag="so")
                        nc.vector.tensor_copy(so[:], po[:])
                        if ch == T // CH - 1:
                            # last chunk: outputs ride the two fast HWDGE
                            # queues so the slow SWDGE drains well before
                            # the end-of-kernel barrier
                            (nc.sync if mo % 2 == 0 else nc.scalar).dma_start(
                                shared_outT[orow, otok], so[:])
                        elif ch == 0:
                            # first chunk rides sync (idle after the input
                            # streams) while gpsimd/scalar finish draining
                            # the routed-phase output burst
                            nc.sync.dma_start(shared_outT[orow, otok], so[:])
                        else:
                            # split with the scalar HWDGE to keep the SWDGE
                            # comfortably under its drain rate
                            (nc.gpsimd if mo % 2 == 0 else nc.scalar).dma_start(
                                shared_outT[orow, otok], so[:])

    nc.compile()
    return nc


_PROGRAM_CACHE: dict = {}


def _get_program(C: int):
    if C not in _PROGRAM_CACHE:
        _PROGRAM_CACHE[C] = _build_program(C)
    return _PROGRAM_CACHE[C]


def _route_like_reference(xf: np.ndarray, router_w: np.ndarray,
                          expert_bias: np.ndarray):
    """Router computed with jax on CPU to bit-match the reference's top-k."""
    import jax
    import jax.numpy as jnp

    cpu = jax.devices("cpu")[0]
    with jax.default_device(cpu):
        xj = jnp.asarray(xf)
        scores = jax.nn.sigmoid(xj @ jnp.asarray(router_w).T)        # (T, E)
        sel = scores + jnp.asarray(expert_bias)
        _, top_idx = jax.lax.top_k(sel, TOPK)                        # (T, K)
        top_sc = jnp.take_along_axis(scores, top_idx, axis=-1)
        top_w = top_sc / (top_sc.sum(-1, keepdims=True) + 1e-9)
        return np.asarray(top_idx), np.asarray(top_w)


def _pack_pairs(vals: np.ndarray, npairs: int) -> np.ndarray:
    """[npairs*2*P, C] fp32 -> e4m3 [npairs*P, 2*C] DoubleRow layout:
    row j*P+p holds [chunk(2j) row p | chunk(2j+1) row p]."""
    Cc = vals.shape[1]
    out = np.empty((npairs * P, 2 * Cc), dtype=E4M3)
    q = np.clip(vals, -240.0, 240.0).astype(E4M3)
    for j in range(npairs):
        out[j * P:(j + 1) * P, :Cc] = q[2 * j * P:(2 * j + 1) * P]
        out[j * P:(j + 1) * P, Cc:] = q[(2 * j + 1) * P:(2 * j + 2) * P]
    return out


def kernel(x, w12, w3, router_w, expert_bias, sw1, sw2, sw3):
    x = np.asarray(x, dtype=np.float32)
    w12 = np.asarray(w12, dtype=np.float32)
    w3 = np.asarray(w3, dtype=np.float32)
    router_w = np.asarray(router_w, dtype=np.float32)
    expert_bias = np.asarray(expert_bias, dtype=np.float32)
    sw1 = np.asarray(sw1, dtype=np.float32)
    sw2 = np.asarray(sw2, dtype=np.float32)
    sw3 = np.asarray(sw3, dtype=np.float32)

    xf = x.reshape(T, D)
    top_idx, top_w = _route_like_reference(xf, router_w, expert_bias)

    # per-expert token lists + combine weights
    idx_list, w_list = [], []
    for e in range(E):
        hit = top_idx == e                      # (T, K)
        tok = np.nonzero(hit.any(axis=1))[0]
        wt = (top_w * hit).sum(axis=1)[tok]     # combine weight per token
        idx_list.append(tok.astype(np.int64))
        w_list.append(wt.astype(np.float32))

    max_n = max(len(i) for i in idx_list)
    # Device capacity policy: cap at C_CORE (the exact mean load for top-2 of
    # 8 experts) and fix up small per-expert overflows on host in fp32.
    C_CORE = 1024
    C_MAX = 1280   # slab size for the imbalanced-routing fallback
    overflow = sum(max(0, len(i) - C_CORE) for i in idx_list)
    if max_n <= C_CORE:
        C = max(P, -(-max_n // P) * P)          # capacity, multiple of 128
        n_launches, host_fix = 1, False
    elif overflow <= 1024:
        C, n_launches, host_fix = C_CORE, 1, True
    else:
        C = C_MAX
        n_launches, host_fix = max(1, -(-max_n // C_MAX)), False

    xTf = np.ascontiguousarray(xf.T)                        # (D, T) fp32
    xT16 = xTf.astype(ml_dtypes.bfloat16)
    # fp8 x at value x*0.25 for the routed expert (all of D)
    xq8 = np.clip(xTf * 0.25, -240, 240).astype(E4M3)       # (D, T)

    nc = _get_program(C)

    outT = np.zeros((D, T), dtype=np.float32)
    global _LAST_RESULTS
    for launch in range(n_launches):
        lo = launch * C_MAX
        in_maps = []
        for c in range(NCORES):
            hs = slice(c * HC, (c + 1) * HC)
            idx_c = idx_list[c][lo:lo + C]
            n_c = len(idx_c)
            xgq_full = np.zeros((D, C), dtype=np.float32)
            if n_c:
                xgq_full[:, :n_c] = xq8[:, idx_c].astype(np.float32)
            xgq = _pack_pairs(xgq_full, NP_U)
            if launch == 0:
                s1 = np.ascontiguousarray(sw1[hs].T).astype(ml_dtypes.bfloat16)
                s2 = np.ascontiguousarray(sw2[hs].T).astype(ml_dtypes.bfloat16)
                s3 = np.ascontiguousarray(sw3[:, hs].T).astype(ml_dtypes.bfloat16)
            else:
                s1 = np.zeros((D, HC), dtype=ml_dtypes.bfloat16)
                s2 = s1
                s3 = np.zeros((HC, D), dtype=ml_dtypes.bfloat16)
            # w12 columns reordered [gate m0-3 | up m0-3 | gate m4-7 |
            # up m4-7]; gate x4, up x2 (x is at 0.25: gate true-scale,
            # u at 0.5 -> h lands at 0.5 for the fp8 down-proj)
            w12t = np.ascontiguousarray(w12[c].T)           # (D, 2HR) fp32
            w12rq = np.concatenate(
                [w12t[:, 0:NT] * 4.0, w12t[:, HR:HR + NT] * 2.0,
                 w12t[:, NT:HR] * 4.0, w12t[:, HR + NT:] * 2.0], axis=1)
            in_maps.append({
                "xT": xT16,
                "sw1T": s1, "sw2T": s2, "sw3T": s3,
                "w12qT": _pack_pairs(w12rq, NP_U),
                "w3qT": _pack_pairs(np.ascontiguousarray(w3[c].T) * 2.0, NP_D),
                "xgqT": xgq,
            })

        res = run_bass_kernel_spmd(nc, in_maps, core_ids=list(range(NCORES)),
                                   **_RUN_KWARGS)
        _LAST_RESULTS = res

        for c in range(NCORES):
            if launch == 0:
                outT += res.results[c]["shared_outT"].astype(np.float32)
            idx_c = idx_list[c][lo:lo + C]
            if len(idx_c):
                ro = res.results[c]["routed_outT"][:, :len(idx_c)].astype(np.float32)
                outT[:, idx_c] += ro * w_list[c][lo:lo + C][None, :]

    if host_fix:
        # fp32 fixup for tokens beyond the device capacity of each expert
        for c in range(NCORES):
            tail = idx_list[c][C:]
            if len(tail) == 0:
                continue
            wts = w_list[c][C:]
            xs = xf[tail]                             # (n, D)
            h12 = xs @ w12[c].T                       # (n, 2*HR)
            h1, h2 = h12[:, :HR], h12[:, HR:]
            h = h1 / (1.0 + np.exp(-h1)) * h2         # silu(h1) * h2
            out = (h * wts[:, None]) @ w3[c].T        # (n, D)
            outT[:, tail] += out.T
    return outT.T.reshape(B, S, D).astype(np.float32)


# test harness hooks: set _RUN_KWARGS = {"trace": True, ...} before calling
# kernel() to profile; read _LAST_RESULTS afterwards.
_RUN_KWARGS: dict = {}
_LAST_RESULTS = None


# revision 31
# speedup vs baseline: 1.1252x; 1.0100x over previous
"""MoE (shared expert + 8 routed experts, top-2) on 8 Trainium2 NeuronCores.

Sharding: core c holds
  - shared-expert slice c: rows [c*1024, (c+1)*1024) of sw1/sw2 and the
    matching columns of sw3  -> partial (T, D) output, summed on host
  - routed expert c's weights (w12[c], w3[c]); host routes/gathers the
    tokens selected for expert c (capacity 1024 = the exact mean load),
    device computes unscaled expert outputs, host applies combine weights
    during the fp32 scatter-add; small per-expert overflows beyond the
    capacity are fixed up on host in fp32.

Precision: the shared expert runs bf16 with fp32 PSUM (its 8-way
H-sharded contraction sums across cores, so even one fp8 DoubleRow pair
per slice puts 1/4 of the global contraction in fp8 — too noisy).  The
ROUTED expert runs entirely in fp8-e4m3 DoubleRow (2x PE throughput):
its per-expert contraction lives on one core, and full-fp8 routed costs
~1.0e-2 of the 2e-2 error budget.  Scale products are exactly 1
(x*0.25 @ w12*4 for gates; x*0.25 @ w12*2 -> u*0.5 -> h*0.5 fp8 from
the DVE mul; h*0.5 @ w3*2), so fp8 partial sums are true-scale and no
dequant exists anywhere.  Simulated end-to-end max-rel 1.62e-2.

Schedule: routed expert first (fp8 weights stream at t=0, phase is now
~2x shorter), shared-expert weights prefetch behind routed compute;
drain copies/DMAs spread across engines at phase boundaries and the
tail; routed outputs split SWDGE/scalar-HWDGE (the fp8 down-proj
outpaces the SWDGE alone).
"""

import sys

if "/opt/trn_rl_repo" not in sys.path:
    sys.path.insert(0, "/opt/trn_rl_repo")

from contextlib import ExitStack

import numpy as np
import ml_dtypes

import concourse.bass as bass
import concourse.tile as tile
from concourse import mybir, bacc
from concourse.bass_utils import run_bass_kernel_spmd

BF16 = mybir.dt.bfloat16
F32 = mybir.dt.float32
F8 = mybir.dt.float8e4
AF = mybir.ActivationFunctionType
DR = mybir.MatmulPerfMode.DoubleRow
E4M3 = ml_dtypes.float8_e4m3

# Problem shape (hardcoded per spec)
B, S, D = 2, 2048, 2048
T = B * S                  # 4096 tokens
E = 8                      # routed experts == n_cores
TOPK = 2
H_SHARED = 8192
HC = H_SHARED // 8         # shared-expert hidden slice per core
HR = 1024                  # routed expert hidden
NCORES = 8
NT = 512                   # token block (one PSUM bank at fp32)
P = 128
CH = 1024                  # shared-phase x chunk (2 sub-blocks)

KD = D // P                # 16 contraction tiles over D
KH = HC // P               # 8 contraction tiles over HC (== HR // P)
NP_U = 8                   # routed up-proj fp8 pairs (all of D)
NP_D = 4                   # routed down-proj fp8 pairs (all of HR)
H2 = HC // 2               # shared up-proj weight half-tile width


def _build_program(C: int):
    """SPMD Bass program, routed capacity C (multiple of 128)."""
    nc = bacc.Bacc("TRN2", target_bir_lowering=False, debug=False)

    xT = nc.dram_tensor("xT", [D, T], BF16, kind="ExternalInput")
    sw1T = nc.dram_tensor("sw1T", [D, HC], BF16, kind="ExternalInput")
    sw2T = nc.dram_tensor("sw2T", [D, HC], BF16, kind="ExternalInput")
    sw3T = nc.dram_tensor("sw3T", [HC, D], BF16, kind="ExternalInput")
    # fp8 routed tensors in DoubleRow pair layout: row j*128+p holds
    # [chunk(2j) | chunk(2j+1)] planes side by side.
    # w12q columns: [gate m0-3 | up m0-3 | gate m4-7 | up m4-7],
    # gate values w*4, up values w*2 (folds the h*0.5 scale)
    w12qT = nc.dram_tensor("w12qT", [NP_U * P, 2 * 2 * HR], F8, kind="ExternalInput")
    w3qT = nc.dram_tensor("w3qT", [NP_D * P, 2 * D], F8, kind="ExternalInput")
    xgqT = nc.dram_tensor("xgqT", [NP_U * P, 2 * C], F8, kind="ExternalInput")

    shared_outT = nc.dram_tensor("shared_outT", [D, T], BF16, kind="ExternalOutput")
    routed_outT = nc.dram_tensor("routed_outT", [D, C], BF16, kind="ExternalOutput")

    # routed token blocks
    blk_w = [NT] * (C // NT)
    if C % NT:
        blk_w.append(C % NT)
    NBLK = len(blk_w)
    early_prefetch = C <= 1280

    with tile.TileContext(nc) as tc:
        with ExitStack() as ctx:
            # pools that live across both phases
            hpool = ctx.enter_context(tc.tile_pool(name="h_p", bufs=2))
            tpool = ctx.enter_context(tc.tile_pool(name="t_p", bufs=2))
            # shared-phase first-half weights + chunk-0/sb-0 x: entered
            # before the routed pools (pool release is LIFO) so they
            # survive into the shared phase
            wsh1 = ctx.enter_context(tc.tile_pool(name="w_sh1", bufs=1))
            xA0 = ctx.enter_context(tc.tile_pool(name="x_a0", bufs=1))
            sw1_h1 = [wsh1.tile([P, H2], BF16, name=f"sw1_{k}_0",
                                tag=f"sw1_{k}_0") for k in range(KD)]
            sw2_h1 = [wsh1.tile([P, H2], BF16, name=f"sw2_{k}_0",
                                tag=f"sw2_{k}_0") for k in range(KD)]
            x00 = [xA0.tile([P, NT], BF16, name=f"xa_{k}", tag=f"xa_{k}")
                   for k in range(KD)]

            # input DMAs stay on the sync queue: the stream is DMA-ring
            # completion-limited (~wire speed), and the scalar queue must
            # stay clear for the silu chain
            def emit_prefetch_a():
                for k in range(KD):
                    nc.sync.dma_start(sw1_h1[k][:], sw1T[k * P:(k + 1) * P, :H2])
                    nc.sync.dma_start(sw2_h1[k][:], sw2T[k * P:(k + 1) * P, :H2])
                for k in range(KD):
                    nc.sync.dma_start(x00[k][:], xT[k * P:(k + 1) * P, 0:NT])

            # HAM warmup: dummy matmuls on memset SBUF keep the PE busy
            # through its cold-clock window while the first weights stream
            wmp = ctx.enter_context(tc.tile_pool(name="wm_p", bufs=1))
            wwm = wmp.tile([P, P], BF16, name="wwm", tag="wwm")
            xwm = wmp.tile([P, NT // 2], BF16, name="xwm", tag="xwm")
            nc.vector.memset(wwm[:], 0)
            nc.gpsimd.memset(xwm[:], 0)

            with ExitStack() as ctx_r:
                # ---------------- Phase R: routed expert (all fp8) -------
                wr = ctx_r.enter_context(tc.tile_pool(name="w_r", bufs=1))
                xgp = ctx_r.enter_context(tc.tile_pool(name="xg_p", bufs=1))
                # deep routed output ring: the fp8 down-proj produces so
                # tiles faster than the SWDGE drains them
                orp = ctx_r.enter_context(tc.tile_pool(name="or_p", bufs=8))
                hqp = ctx_r.enter_context(tc.tile_pool(name="hq_p", bufs=2))
                psR = ctx_r.enter_context(
                    tc.tile_pool(name="psR", bufs=1, space="PSUM"))

                for g in range(2):
                    pw = psR.tile([P, NT], F32, name="pb7", tag="pb7")
                    for k in range(5):
                        nc.tensor.matmul(pw[:, :NT // 2], wwm[:], xwm[:],
                                         start=(k == 0), stop=(k == 4))

                w12q_sb = [wr.tile([P, 2, 2 * HR], F8, name=f"w12q_{j}",
                                   tag=f"w12q_{j}") for j in range(NP_U)]
                w3q_sb = [wr.tile([P, 2, D], F8, name=f"w3q_{j}",
                                  tag=f"w3q_{j}") for j in range(NP_D)]
                xgq_sb = [xgp.tile([P, 2, C], F8, name=f"xgq_{j}",
                                   tag=f"xgq_{j}") for j in range(NP_U)]

                def w12q_src(j):
                    return w12qT[j * P:(j + 1) * P, :].rearrange(
                        "p (i c) -> p i c", i=2)

                def xgq_src(j):
                    return xgqT[j * P:(j + 1) * P, :].rearrange(
                        "p (i c) -> p i c", i=2)

                # DMA emission = execution order on the sync queue,
                # need-ordered: block-0 tokens + mg0 weight halves first
                nt0 = min(NT, C)
                for j in range(NP_U):
                    nc.sync.dma_start(xgq_sb[j][:, :, 0:nt0],
                                      xgq_src(j)[:, :, 0:nt0])
                    nc.sync.dma_start(w12q_sb[j][:, :, 0:HR],
                                      w12q_src(j)[:, :, 0:HR])
                for j in range(NP_U):
                    nc.sync.dma_start(w12q_sb[j][:, :, HR:],
                                      w12q_src(j)[:, :, HR:])
                for j in range(NP_D):
                    nc.sync.dma_start(w3q_sb[j][:, :, :],
                                      w3qT[j * P:(j + 1) * P, :].rearrange(
                                          "p (i c) -> p i c", i=2))
                if C > nt0:
                    for j in range(NP_U):
                        nc.sync.dma_start(xgq_sb[j][:, :, nt0:],
                                          xgq_src(j)[:, :, nt0:])

                if early_prefetch:
                    emit_prefetch_a()

                # routed compute
                off = 0
                for b, nt in enumerate(blk_w):
                    tok = slice(off, off + nt)
                    off += nt
                    # fine-grained first chunks: the first matmuls gate on
                    # one xgq+w12q pair so real work starts sooner
                    jchunks = ([range(0, 1), range(1, 2), range(2, 4),
                                range(4, 8)]
                               if b == 0 else [range(0, 4), range(4, 8)])
                    hq = [hqp.tile([P, 2, NT], F8, name=f"hq_{j}",
                                   tag=f"hq_{j}") for j in range(NP_D)]
                    for mg in range(2):
                        pg = [psR.tile([P, NT], F32, name=f"pb{m}", tag=f"pb{m}")
                              for m in range(4)]
                        pu = [psR.tile([P, NT], F32, name=f"pb{4+m}", tag=f"pb{4+m}")
                              for m in range(4)]
                        for js in jchunks:
                            for m in range(4):
                                c1 = slice(mg * HR + m * P, mg * HR + (m + 1) * P)
                                c2 = slice(mg * HR + NT + m * P,
                                           mg * HR + NT + (m + 1) * P)
                                for j in js:
                                    nc.tensor.matmul(pg[m][:, :nt],
                                                     w12q_sb[j][:, :, c1],
                                                     xgq_sb[j][:, :, tok],
                                                     start=(j == 0),
                                                     stop=(j == NP_U - 1),
                                                     perf_mode=DR)
                                for j in js:
                                    nc.tensor.matmul(pu[m][:, :nt],
                                                     w12q_sb[j][:, :, c2],
                                                     xgq_sb[j][:, :, tok],
                                                     start=(j == 0),
                                                     stop=(j == NP_U - 1),
                                                     perf_mode=DR)
                        for m in range(4):
                            sg = tpool.tile([P, NT], F32, name="sg", tag="sg")
                            nc.scalar.activation(sg[:, :nt], pg[m][:, :nt], AF.Silu)
                            # h (value h*0.5 via the w12 up-scale) written
                            # fp8 into its DoubleRow pair plane
                            hidx = mg * 4 + m
                            nc.vector.tensor_mul(hq[hidx // 2][:, hidx % 2, :nt],
                                                 sg[:, :nt], pu[m][:, :nt])
                    last_blk = (b == NBLK - 1)
                    for mo in range(KD):
                        # On the last block, flip the mo->PSUM-tag map: the
                        # shared phase's first psA tiles reallocate the
                        # last-freed banks, so hand those banks to the
                        # earliest-copied mo tiles
                        pb = (7 - mo % 8) if last_blk else (mo % 8)
                        po = psR.tile([P, NT], F32, name=f"pb{pb}", tag=f"pb{pb}")
                        for j in range(NP_D):
                            nc.tensor.matmul(po[:, :nt],
                                             w3q_sb[j][:, :, mo * P:(mo + 1) * P],
                                             hq[j][:, :, :nt],
                                             start=(j == 0), stop=(j == NP_D - 1),
                                             perf_mode=DR)
                        so = orp.tile([P, NT], BF16, name="so", tag="so")
                        nc.vector.tensor_copy(so[:, :nt], po[:, :nt])
                        # routed outputs split SWDGE/scalar-HWDGE: the fp8
                        # down-proj bursts ~51GB/s of strided DRAM writes,
                        # about 2x what one output queue drains
                        (nc.gpsimd if mo % 2 == 0 else nc.scalar).dma_start(
                            routed_outT[mo * P:(mo + 1) * P, tok], so[:, :nt])

            # ---------------- Phase S: shared expert (bf16) ----------
            # entered after the routed pools freed their SBUF: a deep so
            # ring decouples the PSUM-drain copies from the ~5us strided
            # output-DMA latency (the PSUM po WAR rides the copies)
            opool = ctx.enter_context(tc.tile_pool(name="o_p", bufs=8))
            wsh2 = ctx.enter_context(tc.tile_pool(name="w_sh2", bufs=1))
            wdn = ctx.enter_context(tc.tile_pool(name="w_dn", bufs=1))
            xpool = ctx.enter_context(tc.tile_pool(name="x_p", bufs=2))
            psA = ctx.enter_context(tc.tile_pool(name="psA", bufs=2, space="PSUM"))
            psB = ctx.enter_context(tc.tile_pool(name="psB", bufs=4, space="PSUM"))

            if not early_prefetch:
                emit_prefetch_a()

            # second weight halves, down-proj weights, chunk-0/sb-1 x
            sw1_h2 = [wsh2.tile([P, H2], BF16, name=f"sw1_{k}_1", tag=f"sw1_{k}_1")
                      for k in range(KD)]
            sw2_h2 = [wsh2.tile([P, H2], BF16, name=f"sw2_{k}_1", tag=f"sw2_{k}_1")
                      for k in range(KD)]
            sw3_sb = [wdn.tile([P, D], BF16, name=f"sw3_{k}", tag=f"sw3_{k}")
                      for k in range(KH)]
            x01 = [xpool.tile([P, NT], BF16, name=f"x_{k}_1", tag=f"x_{k}_1")
                   for k in range(KD)]
            for k in range(KD):
                nc.sync.dma_start(sw1_h2[k][:], sw1T[k * P:(k + 1) * P, H2:])
                nc.sync.dma_start(sw2_h2[k][:], sw2T[k * P:(k + 1) * P, H2:])
            for k in range(KH // 2):
                nc.sync.dma_start(sw3_sb[k][:], sw3T[k * P:(k + 1) * P, :])
            for k in range(KD // 2):
                nc.sync.dma_start(x01[k][:], xT[k * P:(k + 1) * P, NT:CH])
            for k in range(KH // 2, KH):
                nc.sync.dma_start(sw3_sb[k][:], sw3T[k * P:(k + 1) * P, :])
            for k in range(KD // 2, KD):
                nc.sync.dma_start(x01[k][:], xT[k * P:(k + 1) * P, NT:CH])

            sw_h = [[sw1_h1, sw1_h2], [sw2_h1, sw2_h2]]

            for ch in range(T // CH):
                if ch == 0:
                    x_sb = [[x00[k], x01[k]] for k in range(KD)]
                else:
                    x_sb = [[xpool.tile([P, NT], BF16, name=f"x_{k}_{h}",
                                        tag=f"x_{k}_{h}")
                             for h in range(CH // NT)] for k in range(KD)]
                    for k in range(KD):
                        for h in range(CH // NT):
                            nc.sync.dma_start(
                                x_sb[k][h][:],
                                xT[k * P:(k + 1) * P,
                                   ch * CH + h * NT:ch * CH + (h + 1) * NT])
                for sb in range(CH // NT):
                    otok = slice(ch * CH + sb * NT, ch * CH + (sb + 1) * NT)
                    hs = []
                    for m in range(KH):
                        wh, wm = divmod(m, H2 // P)   # which weight half-tile
                        mm = slice(wm * P, (wm + 1) * P)
                        pg = psA.tile([P, NT], F32, name="pg", tag="pg")
                        pu = psA.tile([P, NT], F32, name="pu", tag="pu")
                        for k in range(KD):
                            nc.tensor.matmul(pg[:], sw_h[0][wh][k][:, mm],
                                             x_sb[k][sb][:],
                                             start=(k == 0), stop=(k == KD - 1))
                        for k in range(KD):
                            nc.tensor.matmul(pu[:], sw_h[1][wh][k][:, mm],
                                             x_sb[k][sb][:],
                                             start=(k == 0), stop=(k == KD - 1))
                        sg = tpool.tile([P, NT], F32, name="sg", tag="sg")
                        nc.scalar.activation(sg[:], pg[:], AF.Silu)
                        h = hpool.tile([P, NT], BF16, name=f"h_{m}", tag=f"h_{m}")
                        nc.vector.tensor_mul(h[:], sg[:], pu[:])
                        hs.append(h)
                    last_sb = (ch == T // CH - 1 and sb == CH // NT - 1)
                    for mo in range(KD):
                        orow = slice(mo * P, (mo + 1) * P)
                        if last_sb and mo >= KD - 2:
                            # final two tiles in half-width groups: short
                            # post-matmul drain, DMAs over both HWDGE queues
                            dma_eng = [nc.sync, nc.scalar, nc.sync, nc.scalar]
                            for hf in range(2):
                                pi = (mo - (KD - 2)) * 2 + hf
                                cs = slice(hf * (NT // 2), (hf + 1) * (NT // 2))
                                po = psB.tile([P, NT], F32, name="po", tag="po")
                                for k in range(KH):
                                    nc.tensor.matmul(po[:, :NT // 2],
                                                     sw3_sb[k][:, orow],
                                                     hs[k][:, cs],
                                                     start=(k == 0), stop=(k == KH - 1))
                                so = opool.tile([P, NT], BF16, name="so", tag="so")
                                nc.vector.tensor_copy(so[:, :NT // 2],
                                                      po[:, :NT // 2])
                                dma_eng[pi].dma_start(
                                    shared_outT[orow,
                                                otok.start + hf * (NT // 2):
                                                otok.start + (hf + 1) * (NT // 2)],
                                    so[:, :NT // 2])
                            continue
                        po = psB.tile([P, NT], F32, name="po", tag="po")
                        for k in range(KH):
                            nc.tensor.matmul(po[:], sw3_sb[k][:, orow],
                                             hs[k][:],
                                             start=(k == 0), stop=(k == KH - 1))
                        so = opool.tile([P, NT], BF16, name="so", tag="so")
                        nc.vector.tensor_copy(so[:], po[:])
                        if ch == T // CH - 1:
                            # last chunk: outputs ride the two fast HWDGE
                            # queues so the slow SWDGE drains well before
                            # the end-of-kernel barrier
                            (nc.sync if mo % 2 == 0 else nc.scalar).dma_start(
                                shared_outT[orow, otok], so[:])
                        elif ch == 0:
                            # first chunk rides sync (idle after the input
                            # streams) while gpsimd/scalar finish draining
                            # the routed-phase output burst
                            nc.sync.dma_start(shared_outT[orow, otok], so[:])
                        else:
                            # split with the scalar HWDGE to keep the SWDGE
                            # comfortably under its drain rate
                            (nc.gpsimd if mo % 2 == 0 else nc.scalar).dma_start(
                                shared_outT[orow, otok], so[:])

    nc.compile()
    return nc


_PROGRAM_CACHE: dict = {}


def _get_program(C: int):
    if C not in _PROGRAM_CACHE:
        _PROGRAM_CACHE[C] = _build_program(C)
    return _PROGRAM_CACHE[C]


def _route_like_reference(xf: np.ndarray, router_w: np.ndarray,
                          expert_bias: np.ndarray):
    """Router computed with jax on CPU to bit-match the reference's top-k."""
    import jax
    import jax.numpy as jnp

    cpu = jax.devices("cpu")[0]
    with jax.default_device(cpu):
        xj = jnp.asarray(xf)
        scores = jax.nn.sigmoid(xj @ jnp.asarray(router_w).T)        # (T, E)
        sel = scores + jnp.asarray(expert_bias)
        _, top_idx = jax.lax.top_k(sel, TOPK)                        # (T, K)
        top_sc = jnp.take_along_axis(scores, top_idx, axis=-1)
        top_w = top_sc / (top_sc.sum(-1, keepdims=True) + 1e-9)
        return np.asarray(top_idx), np.asarray(top_w)


def _pack_pairs(vals: np.ndarray, npairs: int) -> np.ndarray:
    """[npairs*2*P, C] fp32 -> e4m3 [npairs*P, 2*C] DoubleRow layout:
    row j*P+p holds [chunk(2j) row p | chunk(2j+1) row p]."""
    Cc = vals.shape[1]
    out = np.empty((npairs * P, 2 * Cc), dtype=E4M3)
    q = np.clip(vals, -240.0, 240.0).astype(E4M3)
    for j in range(npairs):
        out[j * P:(j + 1) * P, :Cc] = q[2 * j * P:(2 * j + 1) * P]
        out[j * P:(j + 1) * P, Cc:] = q[(2 * j + 1) * P:(2 * j + 2) * P]
    return out


def kernel(x, w12, w3, router_w, expert_bias, sw1, sw2, sw3):
    x = np.asarray(x, dtype=np.float32)
    w12 = np.asarray(w12, dtype=np.float32)
    w3 = np.asarray(w3, dtype=np.float32)
    router_w = np.asarray(router_w, dtype=np.float32)
    expert_bias = np.asarray(expert_bias, dtype=np.float32)
    sw1 = np.asarray(sw1, dtype=np.float32)
    sw2 = np.asarray(sw2, dtype=np.float32)
    sw3 = np.asarray(sw3, dtype=np.float32)

    xf = x.reshape(T, D)
    top_idx, top_w = _route_like_reference(xf, router_w, expert_bias)

    # per-expert token lists + combine weights
    idx_list, w_list = [], []
    for e in range(E):
        hit = top_idx == e                      # (T, K)
        tok = np.nonzero(hit.any(axis=1))[0]
        wt = (top_w * hit).sum(axis=1)[tok]     # combine weight per token
        idx_list.append(tok.astype(np.int64))
        w_list.append(wt.astype(np.float32))

    max_n = max(len(i) for i in idx_list)
    # Device capacity policy: cap at C_CORE (the exact mean load for top-2 of
    # 8 experts) and fix up small per-expert overflows on host in fp32.
    C_CORE = 1024
    C_MAX = 1280   # slab size for the imbalanced-routing fallback
    overflow = sum(max(0, len(i) - C_CORE) for i in idx_list)
    if max_n <= C_CORE:
        C = max(P, -(-max_n // P) * P)          # capacity, multiple of 128
        n_launches, host_fix = 1, False
    elif overflow <= 1024:
        C, n_launches, host_fix = C_CORE, 1, True
    else:
        C = C_MAX
        n_launches, host_fix = max(1, -(-max_n // C_MAX)), False

    xTf = np.ascontiguousarray(xf.T)                        # (D, T) fp32
    xT16 = xTf.astype(ml_dtypes.bfloat16)
    # fp8 x at value x*0.25 for the routed expert (all of D)
    xq8 = np.clip(xTf * 0.25, -240, 240).astype(E4M3)       # (D, T)

    nc = _get_program(C)

    outT = np.zeros((D, T), dtype=np.float32)
    global _LAST_RESULTS
    for launch in range(n_launches):
        lo = launch * C_MAX
        in_maps = []
        for c in range(NCORES):
            hs = slice(c * HC, (c + 1) * HC)
            idx_c = idx_list[c][lo:lo + C]
            n_c = len(idx_c)
            xgq_full = np.zeros((D, C), dtype=np.float32)
            if n_c:
                xgq_full[:, :n_c] = xq8[:, idx_c].astype(np.float32)
            xgq = _pack_pairs(xgq_full, NP_U)
            if launch == 0:
                s1 = np.ascontiguousarray(sw1[hs].T).astype(ml_dtypes.bfloat16)
                s2 = np.ascontiguousarray(sw2[hs].T).astype(ml_dtypes.bfloat16)
                s3 = np.ascontiguousarray(sw3[:, hs].T).astype(ml_dtypes.bfloat16)
            else:
                s1 = np.zeros((D, HC), dtype=ml_dtypes.bfloat16)
                s2 = s1
                s3 = np.zeros((HC, D), dtype=ml_dtypes.bfloat16)
            # w12 columns reordered [gate m0-3 | up m0-3 | gate m4-7 |
            # up m4-7]; gate x4, up x2 (x is at 0.25: gate true-scale,
            # u at 0.5 -> h lands at 0.5 for the fp8 down-proj)
            w12t = np.ascontiguousarray(w12[c].T)           # (D, 2HR) fp32
            w12rq = np.concatenate(
                [w12t[:, 0:NT] * 4.0, w12t[:, HR:HR + NT] * 2.0,
                 w12t[:, NT:HR] * 4.0, w12t[:, HR + NT:] * 2.0], axis=1)
            in_maps.append({
                "xT": xT16,
                "sw1T": s1, "sw2T": s2, "sw3T": s3,
                "w12qT": _pack_pairs(w12rq, NP_U),
                "w3qT": _pack_pairs(np.ascontiguousarray(w3[c].T) * 2.0, NP_D),
                "xgqT": xgq,
            })

        res = run_bass_kernel_spmd(nc, in_maps, core_ids=list(range(NCORES)),
                                   **_RUN_KWARGS)
        _LAST_RESULTS = res

        for c in range(NCORES):
            if launch == 0:
                outT += res.results[c]["shared_outT"].astype(np.float32)
            idx_c = idx_list[c][lo:lo + C]
            if len(idx_c):
                ro = res.results[c]["routed_outT"][:, :len(idx_c)].astype(np.float32)
                outT[:, idx_c] += ro * w_list[c][lo:lo + C][None, :]

    if host_fix:
        # fp32 fixup for tokens beyond the device capacity of each expert
        for c in range(NCORES):
            tail = idx_list[c][C:]
            if len(tail) == 0:
                continue
            wts = w_list[c][C:]
            xs = xf[tail]                             # (n, D)
            h12 = xs @ w12[c].T                       # (n, 2*HR)
            h1, h2 = h12[:, :HR], h12[:, HR:]
            h = h1 / (1.0 + np.exp(-h1)) * h2         # silu(h1) * h2
            out = (h * wts[:, None]) @ w3[c].T        # (n, D)
            outT[:, tail] += out.T
    return outT.T.reshape(B, S, D).astype(np.float32)


# test harness hooks: set _RUN_KWARGS = {"trace": True, ...} before calling
# kernel() to profile; read _LAST_RESULTS afterwards.
_RUN_KWARGS: dict = {}
_LAST_RESULTS = None
